# revision 13
# baseline (speedup 1.0000x reference)
"""Trainium2 Bass kernel for nn_DensePoseV1ConvXGNSparseHead.

8 layers of submanifold 3x3 conv (gather-GEMM over 9 taps) + GroupNorm(32)
+ ReLU on N=131072 sparse sites, 256->512 then 512->512 channels.

Strategy
--------
Host reconstructs a planar embedding of the points from nbr_idx, packs the
connected components into a padded dense canvas (stride 66), and runs the
conv as dense channel-major fp16 matmuls.  Inactive/pad cells are kept at
exactly 0 by folding an activity mask into the GroupNorm affine, preserving
submanifold semantics.  Canvas rows are split across the 8 cores with an
8-row halo (receptive field of 8 stacked 3x3 convs) - zero inter-core
communication.

v3 (this file): the 3x3 conv runs as 1-D Winograd F(4,3) along canvas rows:
output rows in groups of 4, 6 position-planes V = B4^T x (DVE row combines),
per-position GEMMs over (3 dx taps x ci chunks) with host-transformed
weights W' = G4 w, output rows Y = A4^T U (DVE).  MACs per cell drop from
36 to 18 per 128-out-chunk, which matters because the baseline was at the
power-throttled PE streaming roofline.  GroupNorm statistics stay fp32.
"""

import numpy as np

import concourse.bass as bass
import concourse.tile as tile
from concourse import bacc, mybir
from concourse.bass_utils import run_bass_kernel_spmd

DT = mybir.dt
F16 = DT.float16

N_TAPS = 9
OFFS = [(dy, dx) for dy in (-1, 0, 1) for dx in (-1, 0, 1)]
OFFS_ARR = np.array(OFFS, np.int64)
HALO_ROWS = 8
N_CORES = 8
HID = 512
GSIZE = 16
EPS = 1e-5
CPAD = 128

SW = 66          # canvas row stride (w.max()+2, asserted host-side)
GQ = 7           # group-rows per block
QB = GQ * SW     # 462 quad-cells per block (psum width)
CB = 4 * QB      # 1848 output cells per block
WROWS = 30       # x-window rows per block (4*GQ + 2)
WPITCH = 2192    # x-window plane pitch (fits 4D rearrange span)
VW = QB + 2      # V-plane width incl 1 garbage-safe edge col each side
NPOS = 6

B4T = np.array([
    [4, 0, -5, 0, 1, 0], [0, -4, -4, 1, 1, 0], [0, 4, -4, -1, 1, 0],
    [0, -2, -1, 2, 1, 0], [0, 2, -1, -2, 1, 0], [0, 4, 0, -5, 0, 1]],
    np.float32)
G4 = np.array([
    [1 / 4, 0, 0], [-1 / 6, -1 / 6, -1 / 6], [-1 / 6, 1 / 6, -1 / 6],
    [1 / 24, 1 / 12, 1 / 6], [1 / 24, -1 / 12, 1 / 6], [0, 0, 1]],
    np.float32)
A4T = np.array([
    [1, 1, 1, 1, 1, 0], [0, 1, -1, 2, -2, 0],
    [0, 1, 1, 4, 4, 0], [0, 1, -1, 8, -8, 1]], np.float32)


# ----------------------------------------------------------------- host side

def _embed_points(nbr):
    n = nbr.shape[0]
    assert nbr.shape[1] == N_TAPS
    assert (nbr[:, 4] == np.arange(n)).all(), "tap 4 must be self"
    comp = np.arange(n, dtype=np.int64)
    py = np.zeros(n, np.int64)
    px = np.zeros(n, np.int64)
    edges = []
    for k in range(N_TAPS):
        if k == 4:
            continue
        t = nbr[:, k]
        src = np.flatnonzero(t >= 0)
        edges.append((src, t[src].astype(np.int64), int(OFFS_ARR[k, 0]),
                      int(OFFS_ARR[k, 1])))
    for _ in range(100_000):
        changed = False
        for src, dst, dy, dx in edges:
            bad = comp[src] < comp[dst]
            if bad.any():
                s, d = src[bad], dst[bad]
                order = np.argsort(comp[s], kind="stable")
                s, d = s[order], d[order]
                uniq, first = np.unique(d, return_index=True)
                s, d = s[first], uniq
                comp[d] = comp[s]
                py[d] = py[s] + dy
                px[d] = px[s] + dx
                changed = True
        if not changed:
            break
    else:
        raise RuntimeError("label propagation did not converge")
    for k in range(N_TAPS):
        t = nbr[:, k]
        src = np.flatnonzero(t >= 0)
        dst = t[src]
        ok = ((comp[src] == comp[dst])
              & (py[dst] == py[src] + OFFS_ARR[k, 0])
              & (px[dst] == px[src] + OFFS_ARR[k, 1]))
        if not ok.all():
            raise RuntimeError(f"rulebook inconsistent at tap {k}")
    return comp, py, px


def _build_canvas_map(nbr):
    n = nbr.shape[0]
    comp, py, px = _embed_points(nbr)
    uniq, inv = np.unique(comp, return_inverse=True)
    ncmp = uniq.size
    big = 1 << 60
    miny = np.full(ncmp, big); minx = np.full(ncmp, big)
    maxy = np.full(ncmp, -big); maxx = np.full(ncmp, -big)
    np.minimum.at(miny, inv, py); np.minimum.at(minx, inv, px)
    np.maximum.at(maxy, inv, py); np.maximum.at(maxx, inv, px)
    h = maxy - miny + 1
    w = maxx - minx + 1
    stride = int(w.max()) + 2
    assert stride == SW, f"stride {stride} != {SW}"
    shelf_w = stride - 2

    npts = np.bincount(inv)
    isbig = npts > 1000
    row_off = np.zeros(ncmp, np.int64)
    col_off = np.ones(ncmp, np.int64)
    acc = 0
    for c in np.flatnonzero(isbig):
        row_off[c] = acc
        acc += int(h[c]) + 1
    order = sorted(np.flatnonzero(~isbig), key=lambda c: -int(h[c]))
    shelf_row, shelf_h, xcur = acc, 0, 0
    for c in order:
        if xcur + int(w[c]) > shelf_w:
            shelf_row += shelf_h + 1
            shelf_h, xcur = 0, 0
        if shelf_h == 0:
            shelf_h = int(h[c])
        row_off[c] = shelf_row
        col_off[c] = 1 + xcur
        xcur += int(w[c]) + 1
    if xcur > 0:
        shelf_row += shelf_h + 1
    total_rows = int(shelf_row)
    # rows per core: multiple of 4 so core canvases align to quad groups
    r8 = -(-total_rows // N_CORES)
    r8 = -(-r8 // 4) * 4
    rg = N_CORES * r8 + 2 * HALO_ROWS
    grow = HALO_ROWS + row_off[inv] + (py - miny[inv])
    gcol = col_off[inv] + (px - minx[inv])
    pos = grow * stride + gcol
    occupied = np.zeros(rg * stride, bool)
    if pos.max() >= occupied.size or np.unique(pos).size != n:
        raise RuntimeError("canvas build failed")
    for k in range(N_TAPS):
        if k == 4:
            continue
        occupied[:] = False
        occupied[pos] = True
        dpos = int(OFFS_ARR[k, 0]) * stride + int(OFFS_ARR[k, 1])
        if occupied[pos[nbr[:, k] < 0] + dpos].any():
            raise RuntimeError(f"tap {k}: active cell where rulebook says -1")
    rows_core = r8 + 2 * HALO_ROWS          # 456
    assert rows_core % 4 == 0
    ng = rows_core // 4                      # quad groups per core
    nb = -(-ng // GQ)                        # winograd blocks per core
    m_raw = rows_core * stride               # valid cells per core
    m_out = nb * CB                          # cells written per core
    return pos, dict(stride=stride, r8=r8, rg=rg, m_raw=m_raw,
                     ng=ng, n_blocks=nb, m_out=m_out)


# --------------------------------------------------------------- bass program

def _build_program(meta, layers=8):
    nb = meta["n_blocks"]
    m_raw = meta["m_raw"]
    m_out = meta["m_out"]
    # x buffers: room for the last block's window read + zero slack
    win_end = CPAD + (nb - 1) * CB - SW + WROWS * SW
    padw = max(CPAD + m_out, win_end) + 2 * CPAD
    z0_start = CPAD + m_raw
    nc = bacc.Bacc("TRN2", target_bir_lowering=False, debug=False)

    WTC = NPOS * 3  # 18 weight planes per layer

    x0_d = nc.dram_tensor("x0", (2, 128, padw), F16, kind="ExternalInput")
    w0_d = nc.dram_tensor("w0p", (128, WTC * 2 * HID), F16,
                          kind="ExternalInput")
    wr_d = nc.dram_tensor("wrp", (max(layers - 1, 1), 128, WTC * 4 * HID),
                          F16, kind="ExternalInput")
    acg_d = nc.dram_tensor("acg", (layers, 32, 2048), F16, kind="ExternalInput")
    gm_d = nc.dram_tensor("gm16", (layers, 32, 2048), F16, kind="ExternalInput")
    bc_d = nc.dram_tensor("bc32", (layers, 32, 2048), F16, kind="ExternalInput")
    smask_d = nc.dram_tensor("smask", (128, 128), F16, kind="ExternalInput")
    msk32_d = nc.dram_tensor("msk32", (32, nb * QB), F16,
                             kind="ExternalInput")
    out_d = nc.dram_tensor("out", (4, 128, m_out), DT.float32,
                           kind="ExternalOutput")
    xa_d = nc.dram_tensor("xa", (4, 128, padw), F16, kind="Internal")
    xb_d = nc.dram_tensor("xb", (4, 128, padw), F16, kind="Internal")

    with tile.TileContext(nc) as tc:
        with (
            tc.tile_pool(name="consts", bufs=1) as constp,
            tc.tile_pool(name="wp", bufs=1) as wpool,
            tc.tile_pool(name="lyc", bufs=2) as lycp,
            tc.tile_pool(name="vp", bufs=1) as vpool,
            tc.tile_pool(name="vt", bufs=1) as vtp,
            tc.tile_pool(name="yb", bufs=3) as ypool,
            tc.tile_pool(name="ysq", bufs=2) as ysqpool,
            tc.tile_pool(name="yf", bufs=1) as yfpool,
            tc.tile_pool(name="ot", bufs=1) as otpool,
            tc.tile_pool(name="tt", bufs=1) as ttpool,
            tc.tile_pool(name="tt2", bufs=2) as tt2pool,
            tc.tile_pool(name="psU", bufs=1, space=bass.MemorySpace.PSUM) as psUp,
            tc.tile_pool(name="psT", bufs=2, space=bass.MemorySpace.PSUM) as psTp,
            tc.tile_pool(name="psAB", bufs=1, space=bass.MemorySpace.PSUM) as psABp,
        ):
            smask = constp.tile([128, 128], F16)
            nc.sync.dma_start(smask[:], smask_d.ap())
            xw0 = constp.tile([128, 4, WPITCH], F16, tag="xw0")
            xw1 = constp.tile([128, 4, WPITCH], F16, tag="xw1")
            msk0 = constp.tile([64, QB], F16, tag="msk0")
            msk1 = constp.tile([64, QB], F16, tag="msk1")
            xwt = [xw0, xw1]
            mskt = [msk0, msk1]

            # zero the pads of the internal ping-pong buffers once
            zpad = constp.tile([128, CPAD], F16)
            nc.gpsimd.memset(zpad[:], 0.0)
            for buf in (xa_d, xb_d):
                for ci in range(4):
                    nc.sync.dma_start(buf.ap()[ci, :, 0:CPAD], zpad[:])
                    for z0 in range(z0_start, padw, CPAD):
                        zw = min(CPAD, padw - z0)
                        nc.sync.dma_start(buf.ap()[ci, :, z0:z0 + zw],
                                          zpad[:, 0:zw])

            # V plane tiles: fixed pos tags holding all 4 ci planes; edge
            # cols written once and never again (only reach masked outputs)
            vtiles = {}
            for p in range(NPOS):
                vtiles[p] = vpool.tile([128, 4, VW], F16, tag=f"v{p}",
                                       name=f"v{p}")
            for v in vtiles.values():
                for ci in range(4):
                    nc.gpsimd.memset(v[:, ci, 0:1], 0.0)
                    nc.gpsimd.memset(v[:, ci, VW - 1:VW], 0.0)

            def load_weights(li):
                # 18 (pos,dx) planes; tag-level deps let the next layer's
                # DMAs start as this layer's last reader of a plane retires
                nci = 2 if li == 0 else 4
                wq = nci * 4 * 128
                tiles = []
                for pd in range(WTC):
                    wsb = wpool.tile([128, 4 * 4 * 128], F16, tag=f"w{pd}",
                                     name=f"w{pd}")
                    src = (w0_d.ap() if li == 0 else wr_d.ap()[li - 1])
                    nc.sync.dma_start(wsb[:, 0:wq],
                                      src[:, pd * wq:(pd + 1) * wq])
                    tiles.append(wsb)
                return tiles

            def load_xw(pp, src_aps, nci, bexpr):
                for ci in range(nci):
                    nc.sync.dma_start(
                        xwt[pp][:, ci, 0:WROWS * SW],
                        src_aps[ci][:, bass.ds(bexpr * CB + (CPAD - SW),
                                               WROWS * SW)])
                nc.sync.dma_start(
                    mskt[pp][0:32, :],
                    msk32_d.ap()[:, bass.ds(bexpr * QB, QB)])
                nc.sync.dma_start(
                    mskt[pp][32:64, :],
                    msk32_d.ap()[:, bass.ds(bexpr * QB, QB)])

            TT = mybir.AluOpType

            def run_layer(li, nci, src_aps, dst_aps, final, w_tiles):

                def v_build(cur):
                    xw = xwt[cur]

                    def d(i):
                        # 4D AP [128, nci, GQ, SW]: window rows 4g+i
                        seg = xw[:, 0:nci, i * SW:i * SW + GQ * 4 * SW]
                        return seg.rearrange("p ci (g q c) -> p ci g q c",
                                             g=GQ, q=4)[:, :, :, 0, :]

                    tt = nc.vector.tensor_tensor
                    CP = mybir.ActivationFunctionType.Copy

                    def sc(out, in_, k):
                        nc.scalar.activation(out, in_, CP, scale=float(k))

                    def fl(t):
                        return t[:].rearrange("p (ci g c) -> p ci g c",
                                              ci=4, g=GQ)[:, 0:nci]

                    def vout(p):
                        return vtiles[p][:, 0:nci, 1:1 + QB].rearrange(
                            "p ci (g c) -> p ci g c", g=GQ)

                    d0, d1, d2, d3, d4, d5 = (d(i) for i in range(6))
                    stt = nc.vector.scalar_tensor_tensor
                    CP = mybir.ActivationFunctionType.Copy

                    def sc(out, in_, k):
                        nc.scalar.activation(out, in_, CP, scale=float(k))

                    s1 = vtp.tile([128, 4 * QB], F16, tag="s1", name="vt_s1")
                    s2 = vtp.tile([128, 4 * QB], F16, tag="s2", name="vt_s2")

                    def f3(t):  # 3D view of flat scratch: [p, ci, QB]
                        return t[:].rearrange("p (ci q) -> p ci q",
                                              ci=4)[:, 0:nci]

                    def vo3(p):  # 3D view of V interior: [p, ci, QB]
                        return vtiles[p][:, 0:nci, 1:1 + QB]

                    # V1 = (d1+d2)*(-4) + (d3+d4); V2 = (d1-d2)*4 + (d4-d3)
                    tt(fl(s1), d1, d2, TT.add)
                    tt(fl(s2), d3, d4, TT.add)
                    stt(vo3(1), f3(s1), -4.0, f3(s2), TT.mult, TT.add)
                    tt(fl(s1), d1, d2, TT.subtract)
                    tt(fl(s2), d4, d3, TT.subtract)
                    stt(vo3(2), f3(s1), 4.0, f3(s2), TT.mult, TT.add)
                    # V3 = 2(d3-d1) + (d4-d2); V4 = -2(d3-d1) + (d4-d2)
                    tt(fl(s1), d3, d1, TT.subtract)
                    tt(fl(s2), d4, d2, TT.subtract)
                    stt(vo3(3), f3(s1), 2.0, f3(s2), TT.mult, TT.add)
                    stt(vo3(4), f3(s1), -2.0, f3(s2), TT.mult, TT.add)
                    # V0 = 4 d0 + (-5 d2 + d4);  V5 = 4 d1 + (-5 d3 + d5)
                    sc(fl(s1), d2, -5.0)
                    tt(fl(s2), fl(s1), d4, TT.add)
                    sc(fl(s1), d0, 4.0)
                    tt(vout(0), fl(s1), fl(s2), TT.add)
                    sc(fl(s1), d3, -5.0)
                    tt(fl(s2), fl(s1), d5, TT.add)
                    sc(fl(s1), d1, 4.0)
                    tt(vout(5), fl(s1), fl(s2), TT.add)

                def chain(pos, co, psu):
                    mi = 0
                    nmm = 3 * nci
                    for dx in range(3):
                        wt = w_tiles[pos * 3 + dx]
                        for ci in range(nci):
                            lhsT = wt[:, (co * nci + ci) * 128:
                                      (co * nci + ci) * 128 + 128]
                            rhs = vtiles[pos][:, ci, dx:dx + QB]
                            nc.tensor.matmul(psu[:], lhsT, rhs,
                                             start=(mi == 0),
                                             stop=(mi == nmm - 1))
                            mi += 1

                def conv_transform(co, y):
                    # 6 position chains through 4 psum banks, interleaved
                    # with the A4T output transform so banks recycle
                    tt = nc.vector.tensor_tensor
                    ts = nc.vector.tensor_scalar
                    ua = psUp.tile([128, QB], DT.float32, tag="Ua",
                                   name="psUa")
                    chain(1, co, ua)
                    ub = psUp.tile([128, QB], DT.float32, tag="Ub",
                                   name="psUb")
                    chain(2, co, ub)
                    c1 = otpool.tile([128, QB], DT.float32, tag="oc",
                                     name="ot_c")
                    nc.vector.tensor_copy(c1[:], ua[:])
                    t_s = otpool.tile([128, QB], DT.float32, tag="os",
                                      name="ot_s")
                    tt(t_s[:], c1[:], ub[:], TT.add)
                    t_d = otpool.tile([128, QB], DT.float32, tag="od",
                                      name="ot_d")
                    tt(t_d[:], c1[:], ub[:], TT.subtract)
                    uc = psUp.tile([128, QB], DT.float32, tag="Uc",
                                   name="psUc")
                    chain(3, co, uc)
                    ud = psUp.tile([128, QB], DT.float32, tag="Ud",
                                   name="psUd")
                    chain(4, co, ud)
                    c2 = otpool.tile([128, QB], DT.float32, tag="oc",
                                     name="ot_c2")
                    nc.vector.tensor_copy(c2[:], uc[:])
                    t_t = otpool.tile([128, QB], DT.float32, tag="ost",
                                      name="ot_t")
                    tt(t_t[:], c2[:], ud[:], TT.add)
                    t_u = otpool.tile([128, QB], DT.float32, tag="ou",
                                      name="ot_u")
                    tt(t_u[:], c2[:], ud[:], TT.subtract)
                    u0 = psUp.tile([128, QB], DT.float32, tag="Ua",
                                   name="psU0")
                    chain(0, co, u0)
                    u5 = psUp.tile([128, QB], DT.float32, tag="Ub",
                                   name="psU5")
                    chain(5, co, u5)

                    y4 = y[:].rearrange("p (g r c) -> p g r c", g=GQ, r=4)
                    stt = nc.vector.scalar_tensor_tensor

                    def flq(t):
                        return t[:].rearrange("p (g c) -> p g c", g=GQ)

                    t_a = otpool.tile([128, QB], DT.float32, tag="oa",
                                      name="ot_a")
                    # Y0 = U0 + s + t
                    tt(t_a[:], u0[:], t_s[:], TT.add)
                    tt(y4[:, :, 0, :], flq(t_a), flq(t_t), TT.add)
                    # Y1 = 2u + d
                    stt(y4[:, :, 1, :], flq(t_u), 2.0, flq(t_d),
                        TT.mult, TT.add)
                    # Y2 = 4t + s
                    stt(y4[:, :, 2, :], flq(t_t), 4.0, flq(t_s),
                        TT.mult, TT.add)
                    # Y3 = 8u + d + U5
                    stt(t_a[:], t_u[:], 8.0, t_d[:], TT.mult, TT.add)
                    tt(y4[:, :, 3, :], flq(t_a), flq(u5[:]), TT.add)

                def ep_stats(co, y, ysq, cur):
                    msk = mskt[cur]
                    pst = psTp.tile([64, QB], DT.float32, tag="st",
                                    name="pst")
                    psX = pst[0:32, :]
                    psXX = pst[32:64, :]
                    acg = lycp.tile([32, 512], F16, tag="acg")
                    nc.sync.dma_start(
                        acg[:], acg_d.ap()[li, :, co * 512:(co + 1) * 512])
                    gm = lycp.tile([32, 512], F16, tag="gm")
                    nc.sync.dma_start(
                        gm[:], gm_d.ap()[li, :, co * 512:(co + 1) * 512])
                    bc = lycp.tile([32, 512], F16, tag="bc")
                    nc.sync.dma_start(
                        bc[:], bc_d.ap()[li, :, co * 512:(co + 1) * 512])
                    for j in range(4):
                        nc.tensor.matmul(psX,
                                         smask[:, j * 32:(j + 1) * 32],
                                         y[:, j * QB:(j + 1) * QB],
                                         start=(j == 0), stop=(j == 3))
                    for j in range(4):
                        nc.tensor.matmul(psXX,
                                         smask[:, j * 32:(j + 1) * 32],
                                         ysq[:, j * QB:(j + 1) * QB],
                                         start=(j == 0), stop=(j == 3))

                    sxs = ttpool.tile([32, QB], DT.float32, tag="sxs")
                    nc.vector.tensor_copy(sxs[:], psX)
                    u2 = ttpool.tile([32, QB], DT.float32, tag="u2")
                    nc.vector.scalar_tensor_tensor(u2[:], sxs[:],
                                                   -1.0 / GSIZE, sxs[:],
                                                   TT.mult, TT.mult)
                    v = ttpool.tile([32, QB], DT.float32, tag="v")
                    nc.vector.tensor_tensor(v[:], psXX, u2[:], TT.add)
                    uu = ttpool.tile([32, QB], DT.float32, tag="u")
                    nc.vector.tensor_scalar(uu[:], v[:], 1.0 / GSIZE, EPS,
                                            TT.mult, TT.add)
                    r = ttpool.tile([32, QB], DT.float32, tag="r")
                    nc.vector.reciprocal_approx_fast(r[:], uu[:])
                    inv = ttpool.tile([32, QB], DT.float32, tag="u2")
                    nc.scalar.activation(inv[:], r[:],
                                         mybir.ActivationFunctionType.Sqrt)
                    invm = tt2pool.tile([32, QB], F16, tag="invm")
                    nc.vector.tensor_tensor(invm[:], inv[:], msk[0:32, :],
                                            TT.mult)
                    w32 = tt2pool.tile([32, QB], F16, tag="w32")
                    nc.vector.tensor_tensor(w32[:], sxs[:], invm[:], TT.mult)
                    return invm, w32, msk, acg, gm, bc

                def ep_ab(co, y, invm, w32, msk, acg, gm, bc, bexpr):
                    if final:
                        yout = yfpool.tile([128, CB], DT.float32, tag="yf")
                    else:
                        yout = y  # relu written in place after t1 reads y
                    for j in range(4):
                        cj = j * 128
                        psA = psABp.tile([128, QB], DT.float32, tag="A",
                                         name="psA")
                        nc.tensor.matmul(psA[:], acg[:, cj:cj + 128],
                                         invm[:], start=True, stop=True)
                        psB = psABp.tile([128, QB], DT.float32, tag="B",
                                         name="psB")
                        nc.tensor.matmul(psB[:], bc[:, cj:cj + 128],
                                         msk[0:32, :], start=True,
                                         stop=False)
                        nc.tensor.matmul(psB[:], gm[:, cj:cj + 128],
                                         w32[:], start=False, stop=True)
                        t1 = tt2pool.tile([128, QB], DT.float32, tag="t1")
                        nc.vector.tensor_tensor(
                            t1[:], psA[:], y[:, j * QB:(j + 1) * QB],
                            TT.mult)
                        t2 = tt2pool.tile([128, QB], DT.float32, tag="t2")
                        nc.vector.tensor_tensor(t2[:], psB[:], t1[:], TT.add)
                        nc.scalar.activation(
                            yout[:, j * QB:(j + 1) * QB], t2[:],
                            mybir.ActivationFunctionType.Relu)

                    dst = dst_aps[co][:, bass.ds(bexpr * CB + (0 if final
                                                              else CPAD),
                                                 CB)]
                    nc.sync.dma_start(dst, yout[:])

                def run_block(bexpr, pre_bexpr, cur):
                    load_xw(1 - cur, src_aps, nci, pre_bexpr)
                    v_build(cur)
                    pstat = []
                    pab = []
                    for co in range(4):
                        y = ypool.tile([128, CB], F16, tag="y")
                        conv_transform(co, y)
                        ysq = ysqpool.tile([128, CB], F16, tag="ysq")
                        nc.vector.tensor_tensor(ysq[:], y[:], y[:], TT.mult)
                        pstat.append((co, y, ysq))
                        if len(pstat) > 1:
                            c_, y_, ysq_ = pstat.pop(0)
                            st = ep_stats(c_, y_, ysq_, cur)
                            pab.append((c_, y_) + st)
                        if len(pab) > 1:
                            ep_ab(*pab.pop(0), bexpr)
                    while pstat:
                        c_, y_, ysq_ = pstat.pop(0)
                        st = ep_stats(c_, y_, ysq_, cur)
                        pab.append((c_, y_) + st)
                        while len(pab) > 1:
                            ep_ab(*pab.pop(0), bexpr)
                    while pab:
                        ep_ab(*pab.pop(0), bexpr)

                cur = 0
                load_xw(0, src_aps, nci, 0)
                nstep = 4
                nbe = (nb - 1) - ((nb - 1) % nstep)
                if nbe:
                    with tc.For_i(0, nbe, nstep,
                                  hint_engines=(mybir.EngineType.PE,)) as i:
                        for u in range(nstep):
                            run_block(i + u, i + u + 1, cur)
                            cur = 1 - cur
                for t in range(nbe, nb):
                    run_block(t, t + 1 if t + 1 < nb else t, cur)
                    cur = 1 - cur
                w_next = (load_weights(li + 1) if li + 1 < layers else None)
                tc.strict_bb_all_engine_barrier()
                return w_next

            bufs = {"x0": x0_d, "xa": xa_d, "xb": xb_d}
            seq = ["x0"] + ["xa", "xb"] * 4
            w_tiles = load_weights(0)
            for li in range(layers):
                src, dst = seq[li], seq[li + 1]
                nci = 2 if li == 0 else 4
                src_aps = [bufs[src].ap()[ci] for ci in range(nci)]
                final = li == layers - 1
                dst_aps = ([out_d.ap()[co] for co in range(4)] if final
                           else [bufs[dst].ap()[co] for co in range(4)])
                w_tiles = run_layer(li, nci, src_aps, dst_aps, final, w_tiles)

    nc.compile()
    return nc


# ------------------------------------------------------------- host packing

def _pack_host(inputs, pos, meta, layers=8):
    feats = np.ascontiguousarray(np.asarray(inputs["features"], np.float32))
    w0 = np.asarray(inputs["w0"], np.float32)
    w_rest = np.asarray(inputs["w_rest"], np.float32)
    gamma = np.asarray(inputs["gamma"], np.float32)
    beta = np.asarray(inputs["beta"], np.float32)
    n, cin = feats.shape
    stride, r8 = meta["stride"], meta["r8"]
    m_raw, m_out, nb = meta["m_raw"], meta["m_out"], meta["n_blocks"]
    rgst = meta["rg"] * stride
    win_end = CPAD + (nb - 1) * CB - SW + WROWS * SW
    padw = max(CPAD + m_out, win_end) + 2 * CPAD

    x_g = np.zeros((cin, rgst), np.float16)
    x_g[:, pos] = feats.T.astype(np.float16)
    mask_g = np.zeros(rgst, np.float16)
    mask_g[pos] = 1.0

    # winograd-transformed weights: per (pos,dx) plane, cols (co, ci, ch)
    def pack_w(w, nci):
        # w: [9, Cin, 512] -> out [128, 18 * nci*4*128]
        cin_ = nci * 128
        planes = []
        for p in range(NPOS):
            for dx in range(3):
                wp = np.zeros((cin_, HID), np.float32)
                for dy in range(3):
                    wp += G4[p, dy] * w[3 * dy + dx]
                # lhsT chunks [128, 128] per (co, ci): part dim = ci part
                arr = wp.reshape(nci, 128, 4, 128).transpose(1, 2, 0, 3)
                # arr[p_part, co, ci, ch]
                planes.append(arr.reshape(128, nci * 4 * 128))
        return np.concatenate(planes, axis=1).astype(np.float16)

    w0p = pack_w(w0, 2)
    nl = max(layers - 1, 1)
    wrp = np.zeros((nl, 128, NPOS * 3 * 4 * HID), np.float16)
    for li in range(layers - 1):
        wrp[li] = pack_w(w_rest[li], 4)

    ch = np.arange(128)
    acg = np.zeros((layers, 32, 4, 4, 128), np.float32)
    gm16 = np.zeros((layers, 32, 4, 4, 128), np.float32)
    bc32 = np.zeros((layers, 32, 4, 4, 128), np.float32)
    for li in range(layers):
        for co in range(4):
            g_ = gamma[li, co * 128:(co + 1) * 128]
            b_ = beta[li, co * 128:(co + 1) * 128]
            for j in range(4):
                rows = 8 * j + ch // GSIZE
                acg[li, rows, co, j, ch] = g_
                gm16[li, rows, co, j, ch] = -g_ / GSIZE
                bc32[li, 8 * j, co, j, :] = b_
    acg = acg.reshape(layers, 32, 2048).astype(np.float16)
    gm16 = gm16.reshape(layers, 32, 2048).astype(np.float16)
    bc32 = bc32.reshape(layers, 32, 2048).astype(np.float16)

    smask = np.zeros((128, 4, 32), np.float16)
    for j in range(4):
        smask[ch, j, 8 * j + ch // GSIZE] = 1.0
    smask = smask.reshape(128, 128)

    in_maps = []
    for s in range(N_CORES):
        c0 = s * r8 * stride
        x0 = np.zeros((2, 128, padw), np.float16)
        seg = x_g[:, c0:min(c0 + m_raw, rgst)]
        x0[:, :, CPAD:CPAD + seg.shape[1]] = seg.reshape(2, 128, -1)
        mc = np.zeros(nb * CB, np.float16)
        mseg = mask_g[c0:min(c0 + m_raw, rgst)]
        mc[:mseg.shape[0]] = mseg
        # msk32[8j+g, b*QB + c] = mask[b*CB + j*QB + c]
        m4 = mc.reshape(nb, 4, QB)
        msk32 = np.zeros((32, nb * QB), np.float16)
        for j in range(4):
            for g in range(8):
                msk32[8 * j + g] = m4[:, j, :].reshape(-1)
        in_maps.append({
            "x0": x0, "w0p": w0p, "wrp": wrp, "acg": acg, "gm16": gm16,
            "bc32": bc32, "smask": smask, "msk32": msk32,
        })
    return in_maps


TRACE = False
LAST_RESULT = {}


def kernel(**inputs) -> np.ndarray:
    nbr = np.asarray(inputs["nbr_idx"])
    n = nbr.shape[0]
    pos, meta = _build_canvas_map(nbr)
    in_maps = _pack_host(inputs, pos, meta)
    nc = _build_program(meta)
    res = run_bass_kernel_spmd(nc, in_maps, list(range(N_CORES)), trace=TRACE)
    LAST_RESULT["exec_time_ns"] = res.exec_time_ns
    LAST_RESULT["profile_json"] = res.profile_json

    stride, r8 = meta["stride"], meta["r8"]
    row = pos // stride
    own = np.clip((row - HALO_ROWS) // r8, 0, N_CORES - 1)
    result = np.zeros((n, HID), np.float32)
    for s in range(N_CORES):
        sel = own == s
        local = pos[sel] - s * r8 * stride
        o = res.results[s]["out"]  # [4, 128, m_out]
        result[sel] = o[:, :, local].reshape(HID, -1).T
    return result


if __name__ == "__main__":
    import reference

    inputs = reference.setup_inputs()
    out = kernel(**{k: np.asarray(v) for k, v in inputs.items()})
    exp = np.asarray(reference.reference(**inputs))
    err = np.linalg.norm(out - exp) / np.linalg.norm(exp)
    print(f"l2 rel err: {err:.3e}")


# revision 17
# speedup vs baseline: 1.0491x; 1.0491x over previous
"""Trainium2 Bass kernel for nn_DensePoseV1ConvXGNSparseHead.

8 layers of submanifold 3x3 conv (gather-GEMM over 9 taps) + GroupNorm(32)
+ ReLU on N=131072 sparse sites, 256->512 then 512->512 channels.

Strategy
--------
Host reconstructs a planar embedding of the points from nbr_idx, packs the
connected components into a padded dense canvas (stride 66), and runs the
conv as dense channel-major fp16 matmuls.  Inactive/pad cells are kept at
exactly 0 by folding an activity mask into the GroupNorm affine, preserving
submanifold semantics.  Canvas rows are split across the 8 cores with an
8-row halo (receptive field of 8 stacked 3x3 convs) - zero inter-core
communication.

v3 (this file): the 3x3 conv runs as 1-D Winograd F(4,3) along canvas rows:
output rows in groups of 4, 6 position-planes V = B4^T x (DVE row combines),
per-position GEMMs over (3 dx taps x ci chunks) with host-transformed
weights W' = G4 w, output rows Y = A4^T U (DVE).  MACs per cell drop from
36 to 18 per 128-out-chunk, which matters because the baseline was at the
power-throttled PE streaming roofline.  GroupNorm statistics stay fp32.
"""

import numpy as np

import concourse.bass as bass
import concourse.tile as tile
from concourse import bacc, mybir
from concourse.bass_utils import run_bass_kernel_spmd

DT = mybir.dt
F16 = DT.float16

N_TAPS = 9
OFFS = [(dy, dx) for dy in (-1, 0, 1) for dx in (-1, 0, 1)]
OFFS_ARR = np.array(OFFS, np.int64)
HALO_ROWS = 8
N_CORES = 8
HID = 512
GSIZE = 16
EPS = 1e-5
CPAD = 128

SW = 66          # canvas row stride (w.max()+2, asserted host-side)
GQ = 7           # group-rows per block
QB = GQ * SW     # 462 quad-cells per block (psum width)
CB = 4 * QB      # 1848 output cells per block
WROWS = 30       # x-window rows per block (4*GQ + 2)
WPITCH = 2192    # x-window plane pitch (fits 4D rearrange span)
VW = QB + 2      # V-plane width incl 1 garbage-safe edge col each side
NPOS = 6

B4T = np.array([
    [4, 0, -5, 0, 1, 0], [0, -4, -4, 1, 1, 0], [0, 4, -4, -1, 1, 0],
    [0, -2, -1, 2, 1, 0], [0, 2, -1, -2, 1, 0], [0, 4, 0, -5, 0, 1]],
    np.float32)
G4 = np.array([
    [1 / 4, 0, 0], [-1 / 6, -1 / 6, -1 / 6], [-1 / 6, 1 / 6, -1 / 6],
    [1 / 24, 1 / 12, 1 / 6], [1 / 24, -1 / 12, 1 / 6], [0, 0, 1]],
    np.float32)
A4T = np.array([
    [1, 1, 1, 1, 1, 0], [0, 1, -1, 2, -2, 0],
    [0, 1, 1, 4, 4, 0], [0, 1, -1, 8, -8, 1]], np.float32)


# ----------------------------------------------------------------- host side

def _embed_points(nbr):
    n = nbr.shape[0]
    assert nbr.shape[1] == N_TAPS
    assert (nbr[:, 4] == np.arange(n)).all(), "tap 4 must be self"
    comp = np.arange(n, dtype=np.int64)
    py = np.zeros(n, np.int64)
    px = np.zeros(n, np.int64)
    edges = []
    for k in range(N_TAPS):
        if k == 4:
            continue
        t = nbr[:, k]
        src = np.flatnonzero(t >= 0)
        edges.append((src, t[src].astype(np.int64), int(OFFS_ARR[k, 0]),
                      int(OFFS_ARR[k, 1])))
    for _ in range(100_000):
        changed = False
        for src, dst, dy, dx in edges:
            bad = comp[src] < comp[dst]
            if bad.any():
                s, d = src[bad], dst[bad]
                order = np.argsort(comp[s], kind="stable")
                s, d = s[order], d[order]
                uniq, first = np.unique(d, return_index=True)
                s, d = s[first], uniq
                comp[d] = comp[s]
                py[d] = py[s] + dy
                px[d] = px[s] + dx
                changed = True
        if not changed:
            break
    else:
        raise RuntimeError("label propagation did not converge")
    for k in range(N_TAPS):
        t = nbr[:, k]
        src = np.flatnonzero(t >= 0)
        dst = t[src]
        ok = ((comp[src] == comp[dst])
              & (py[dst] == py[src] + OFFS_ARR[k, 0])
              & (px[dst] == px[src] + OFFS_ARR[k, 1]))
        if not ok.all():
            raise RuntimeError(f"rulebook inconsistent at tap {k}")
    return comp, py, px


def _build_canvas_map(nbr):
    n = nbr.shape[0]
    comp, py, px = _embed_points(nbr)
    uniq, inv = np.unique(comp, return_inverse=True)
    ncmp = uniq.size
    big = 1 << 60
    miny = np.full(ncmp, big); minx = np.full(ncmp, big)
    maxy = np.full(ncmp, -big); maxx = np.full(ncmp, -big)
    np.minimum.at(miny, inv, py); np.minimum.at(minx, inv, px)
    np.maximum.at(maxy, inv, py); np.maximum.at(maxx, inv, px)
    h = maxy - miny + 1
    w = maxx - minx + 1
    stride = int(w.max()) + 2
    assert stride == SW, f"stride {stride} != {SW}"
    shelf_w = stride - 2

    npts = np.bincount(inv)
    isbig = npts > 1000
    row_off = np.zeros(ncmp, np.int64)
    col_off = np.ones(ncmp, np.int64)
    acc = 0
    for c in np.flatnonzero(isbig):
        row_off[c] = acc
        acc += int(h[c]) + 1
    order = sorted(np.flatnonzero(~isbig), key=lambda c: -int(h[c]))
    shelf_row, shelf_h, xcur = acc, 0, 0
    for c in order:
        if xcur + int(w[c]) > shelf_w:
            shelf_row += shelf_h + 1
            shelf_h, xcur = 0, 0
        if shelf_h == 0:
            shelf_h = int(h[c])
        row_off[c] = shelf_row
        col_off[c] = 1 + xcur
        xcur += int(w[c]) + 1
    if xcur > 0:
        shelf_row += shelf_h + 1
    total_rows = int(shelf_row)
    # rows per core: multiple of 4 so core canvases align to quad groups
    r8 = -(-total_rows // N_CORES)
    r8 = -(-r8 // 4) * 4
    rg = N_CORES * r8 + 2 * HALO_ROWS
    grow = HALO_ROWS + row_off[inv] + (py - miny[inv])
    gcol = col_off[inv] + (px - minx[inv])
    pos = grow * stride + gcol
    occupied = np.zeros(rg * stride, bool)
    if pos.max() >= occupied.size or np.unique(pos).size != n:
        raise RuntimeError("canvas build failed")
    for k in range(N_TAPS):
        if k == 4:
            continue
        occupied[:] = False
        occupied[pos] = True
        dpos = int(OFFS_ARR[k, 0]) * stride + int(OFFS_ARR[k, 1])
        if occupied[pos[nbr[:, k] < 0] + dpos].any():
            raise RuntimeError(f"tap {k}: active cell where rulebook says -1")
    rows_core = r8 + 2 * HALO_ROWS          # 456
    assert rows_core % 4 == 0
    ng = rows_core // 4                      # quad groups per core
    nf = ng // GQ                            # full winograd blocks per core
    gt = ng - nf * GQ                        # tail block group-rows
    m_raw = rows_core * stride               # valid cells per core
    return pos, dict(stride=stride, r8=r8, rg=rg, m_raw=m_raw,
                     ng=ng, n_full=nf, g_tail=gt, m_out=m_raw)


# --------------------------------------------------------------- bass program

def _build_program(meta, layers=8):
    nf = meta["n_full"]
    gt = meta["g_tail"]
    ng = meta["ng"]
    m_raw = meta["m_raw"]
    m_out = meta["m_out"]
    # x buffers: room for the last (full-width) window read + zero slack
    win_end = CPAD + nf * CB - SW + WROWS * SW
    padw = max(CPAD + m_out, win_end) + 2 * CPAD
    z0_start = CPAD + m_raw
    nc = bacc.Bacc("TRN2", target_bir_lowering=False, debug=False)

    WTC = NPOS * 3  # 18 weight planes per layer

    x0_d = nc.dram_tensor("x0", (2, 128, padw), F16, kind="ExternalInput")
    w0_d = nc.dram_tensor("w0p", (128, WTC * 2 * HID), F16,
                          kind="ExternalInput")
    wr_d = nc.dram_tensor("wrp", (max(layers - 1, 1), 128, WTC * 4 * HID),
                          F16, kind="ExternalInput")
    acg_d = nc.dram_tensor("acg", (layers, 32, 2048), F16, kind="ExternalInput")
    gm_d = nc.dram_tensor("gm16", (layers, 32, 2048), F16, kind="ExternalInput")
    bc_d = nc.dram_tensor("bc32", (layers, 32, 2048), F16, kind="ExternalInput")
    smask_d = nc.dram_tensor("smask", (128, 128), F16, kind="ExternalInput")
    msk32_d = nc.dram_tensor("msk32", (32, (nf + 1) * QB), F16,
                             kind="ExternalInput")
    out_d = nc.dram_tensor("out", (4, 128, m_out), DT.float32,
                           kind="ExternalOutput")
    xa_d = nc.dram_tensor("xa", (4, 128, padw), F16, kind="Internal")
    xb_d = nc.dram_tensor("xb", (4, 128, padw), F16, kind="Internal")

    with tile.TileContext(nc) as tc:
        with (
            tc.tile_pool(name="consts", bufs=1) as constp,
            tc.tile_pool(name="wp", bufs=1) as wpool,
            tc.tile_pool(name="lyc", bufs=2) as lycp,
            tc.tile_pool(name="vp", bufs=1) as vpool,
            tc.tile_pool(name="vt", bufs=1) as vtp,
            tc.tile_pool(name="yb", bufs=3) as ypool,
            tc.tile_pool(name="ysq", bufs=2) as ysqpool,
            tc.tile_pool(name="yf", bufs=1) as yfpool,
            tc.tile_pool(name="ot", bufs=1) as otpool,
            tc.tile_pool(name="tt", bufs=1) as ttpool,
            tc.tile_pool(name="tt2", bufs=2) as tt2pool,
            tc.tile_pool(name="psU", bufs=1, space=bass.MemorySpace.PSUM) as psUp,
            tc.tile_pool(name="psT", bufs=2, space=bass.MemorySpace.PSUM) as psTp,
            tc.tile_pool(name="psA2", bufs=2, space=bass.MemorySpace.PSUM) as psAp2,
            tc.tile_pool(name="psB1", bufs=1, space=bass.MemorySpace.PSUM) as psBp1,
        ):
            smask = constp.tile([128, 128], F16)
            nc.sync.dma_start(smask[:], smask_d.ap())
            xw0 = constp.tile([128, 4, WPITCH], F16, tag="xw0")
            xw1 = constp.tile([128, 4, WPITCH], F16, tag="xw1")
            msk0 = constp.tile([64, QB], F16, tag="msk0")
            msk1 = constp.tile([64, QB], F16, tag="msk1")
            xwt = [xw0, xw1]
            mskt = [msk0, msk1]

            # zero the pads of the internal ping-pong buffers once
            zpad = constp.tile([128, CPAD], F16)
            nc.gpsimd.memset(zpad[:], 0.0)
            for buf in (xa_d, xb_d):
                for ci in range(4):
                    nc.sync.dma_start(buf.ap()[ci, :, 0:CPAD], zpad[:])
                    for z0 in range(z0_start, padw, CPAD):
                        zw = min(CPAD, padw - z0)
                        nc.sync.dma_start(buf.ap()[ci, :, z0:z0 + zw],
                                          zpad[:, 0:zw])

            # V plane tiles: fixed pos tags holding all 4 ci planes; edge
            # cols written once and never again (only reach masked outputs)
            vtiles = {}
            for p in range(NPOS):
                vtiles[p] = vpool.tile([128, 4, VW], F16, tag=f"v{p}",
                                       name=f"v{p}")
            for v in vtiles.values():
                for ci in range(4):
                    nc.gpsimd.memset(v[:, ci, 0:1], 0.0)
                    nc.gpsimd.memset(v[:, ci, VW - 1:VW], 0.0)

            def load_weights(li):
                # 18 (pos,dx) planes; tag-level deps let the next layer's
                # DMAs start as this layer's last reader of a plane retires
                nci = 2 if li == 0 else 4
                wq = nci * 4 * 128
                tiles = []
                for pd in range(WTC):
                    wsb = wpool.tile([128, 4 * 4 * 128], F16, tag=f"w{pd}",
                                     name=f"w{pd}")
                    src = (w0_d.ap() if li == 0 else wr_d.ap()[li - 1])
                    nc.sync.dma_start(wsb[:, 0:wq],
                                      src[:, pd * wq:(pd + 1) * wq])
                    tiles.append(wsb)
                return tiles

            def load_xw(pp, src_aps, nci, bexpr):
                for ci in range(nci):
                    nc.sync.dma_start(
                        xwt[pp][:, ci, 0:WROWS * SW],
                        src_aps[ci][:, bass.ds(bexpr * CB + (CPAD - SW),
                                               WROWS * SW)])
                nc.sync.dma_start(
                    mskt[pp][0:32, :],
                    msk32_d.ap()[:, bass.ds(bexpr * QB, QB)])
                nc.sync.dma_start(
                    mskt[pp][32:64, :],
                    msk32_d.ap()[:, bass.ds(bexpr * QB, QB)])

            TT = mybir.AluOpType
            SQ = mybir.ActivationFunctionType.Square
            CPF = mybir.ActivationFunctionType.Copy

            def run_layer(li, nci, src_aps, dst_aps, final, w_tiles):

                def load_xw2(pp, bexpr, gq):
                    wr = 4 * gq + 2
                    for ci in range(nci):
                        nc.sync.dma_start(
                            xwt[pp][:, ci, 0:wr * SW],
                            src_aps[ci][:, bass.ds(bexpr * CB + (CPAD - SW),
                                                   wr * SW)])
                    qb = gq * SW
                    nc.sync.dma_start(
                        mskt[pp][0:32, 0:qb],
                        msk32_d.ap()[:, bass.ds(bexpr * QB, qb)])
                    nc.sync.dma_start(
                        mskt[pp][32:64, 0:qb],
                        msk32_d.ap()[:, bass.ds(bexpr * QB, qb)])

                def v_build(cur, gq):
                    xw = xwt[cur]
                    qb = gq * SW

                    def d(i):
                        seg = xw[:, 0:nci, i * SW:i * SW + gq * 4 * SW]
                        return seg.rearrange("p ci (g q c) -> p ci g q c",
                                             g=gq, q=4)[:, :, :, 0, :]

                    tt = nc.vector.tensor_tensor
                    stt = nc.vector.scalar_tensor_tensor

                    def sc(out, in_, k):
                        nc.scalar.activation(out, in_, CPF, scale=float(k))

                    def fl(t):
                        return t[:, 0:4 * qb].rearrange(
                            "p (ci g c) -> p ci g c",
                            ci=4, g=gq)[:, 0:nci]

                    def f3(t):
                        return t[:, 0:4 * qb].rearrange(
                            "p (ci q) -> p ci q", ci=4)[:, 0:nci]

                    def vout(p):
                        return vtiles[p][:, 0:nci, 1:1 + qb].rearrange(
                            "p ci (g c) -> p ci g c", g=gq)

                    def vo3(p):
                        return vtiles[p][:, 0:nci, 1:1 + qb]

                    d0, d1, d2, d3, d4, d5 = (d(i) for i in range(6))
                    s1 = vtp.tile([128, 4 * QB], F16, tag="s1", name="vt_s1")
                    s2 = vtp.tile([128, 4 * QB], F16, tag="s2", name="vt_s2")
                    # V1 = (d1+d2)*(-4) + (d3+d4); V2 = (d1-d2)*4 + (d4-d3)
                    tt(fl(s1), d1, d2, TT.add)
                    tt(fl(s2), d3, d4, TT.add)
                    stt(vo3(1), f3(s1), -4.0, f3(s2), TT.mult, TT.add)
                    tt(fl(s1), d1, d2, TT.subtract)
                    tt(fl(s2), d4, d3, TT.subtract)
                    stt(vo3(2), f3(s1), 4.0, f3(s2), TT.mult, TT.add)
                    # V3 = 2(d3-d1) + (d4-d2); V4 = -2(d3-d1) + (d4-d2)
                    tt(fl(s1), d3, d1, TT.subtract)
                    tt(fl(s2), d4, d2, TT.subtract)
                    stt(vo3(3), f3(s1), 2.0, f3(s2), TT.mult, TT.add)
                    stt(vo3(4), f3(s1), -2.0, f3(s2), TT.mult, TT.add)
                    # V0 = 4 d0 + (-5 d2 + d4);  V5 = 4 d1 + (-5 d3 + d5)
                    sc(fl(s1), d2, -5.0)
                    tt(fl(s2), fl(s1), d4, TT.add)
                    sc(fl(s1), d0, 4.0)
                    tt(vout(0), fl(s1), fl(s2), TT.add)
                    sc(fl(s1), d3, -5.0)
                    tt(fl(s2), fl(s1), d5, TT.add)
                    sc(fl(s1), d1, 4.0)
                    tt(vout(5), fl(s1), fl(s2), TT.add)

                def chain(pos, co, psu, qb):
                    mi = 0
                    nmm = 3 * nci
                    for dx in range(3):
                        wt = w_tiles[pos * 3 + dx]
                        for ci in range(nci):
                            lhsT = wt[:, (co * nci + ci) * 128:
                                      (co * nci + ci) * 128 + 128]
                            rhs = vtiles[pos][:, ci, dx:dx + qb]
                            nc.tensor.matmul(psu[:, 0:qb], lhsT, rhs,
                                             start=(mi == 0),
                                             stop=(mi == nmm - 1))
                            mi += 1

                def conv_transform(co, y, gq):
                    # 6 position chains through 3 psum banks, interleaved
                    # with the A4T output transform so banks recycle
                    qb = gq * SW
                    tt = nc.vector.tensor_tensor
                    stt = nc.vector.scalar_tensor_tensor
                    ua = psUp.tile([128, QB], DT.float32, tag="Ua",
                                   name="psUa")
                    chain(1, co, ua, qb)
                    ub = psUp.tile([128, QB], DT.float32, tag="Ub",
                                   name="psUb")
                    chain(2, co, ub, qb)
                    c1 = otpool.tile([128, QB], DT.float32, tag="oc",
                                     name="ot_c")
                    nc.vector.tensor_copy(c1[:, 0:qb], ua[:, 0:qb])
                    t_s = otpool.tile([128, QB], DT.float32, tag="os",
                                      name="ot_s")
                    tt(t_s[:, 0:qb], c1[:, 0:qb], ub[:, 0:qb], TT.add)
                    t_d = otpool.tile([128, QB], DT.float32, tag="od",
                                      name="ot_d")
                    tt(t_d[:, 0:qb], c1[:, 0:qb], ub[:, 0:qb], TT.subtract)
                    uc = psUp.tile([128, QB], DT.float32, tag="Ua",
                                   name="psUc")
                    chain(3, co, uc, qb)
                    ud = psUp.tile([128, QB], DT.float32, tag="Uc",
                                   name="psUd")
                    chain(4, co, ud, qb)
                    c2 = otpool.tile([128, QB], DT.float32, tag="oc",
                                     name="ot_c2")
                    nc.vector.tensor_copy(c2[:, 0:qb], uc[:, 0:qb])
                    t_t = otpool.tile([128, QB], DT.float32, tag="ost",
                                      name="ot_t")
                    tt(t_t[:, 0:qb], c2[:, 0:qb], ud[:, 0:qb], TT.add)
                    t_u = otpool.tile([128, QB], DT.float32, tag="ou",
                                      name="ot_u")
                    tt(t_u[:, 0:qb], c2[:, 0:qb], ud[:, 0:qb], TT.subtract)
                    u0 = psUp.tile([128, QB], DT.float32, tag="Ub",
                                   name="psU0")
                    chain(0, co, u0, qb)
                    u5 = psUp.tile([128, QB], DT.float32, tag="Ua",
                                   name="psU5")
                    chain(5, co, u5, qb)

                    y4 = y[:, 0:4 * qb].rearrange("p (g r c) -> p g r c",
                                                  g=gq, r=4)

                    def flq(ap):
                        return ap[:, 0:qb].rearrange("p (g c) -> p g c", g=gq)

                    t_a = otpool.tile([128, QB], DT.float32, tag="oa",
                                      name="ot_a")
                    # Y0 = U0 + s + t
                    tt(t_a[:, 0:qb], u0[:, 0:qb], t_s[:, 0:qb], TT.add)
                    tt(y4[:, :, 0, :], flq(t_a), flq(t_t), TT.add)
                    # Y1 = 2u + d
                    stt(y4[:, :, 1, :], flq(t_u), 2.0, flq(t_d),
                        TT.mult, TT.add)
                    # Y2 = 4t + s
                    stt(y4[:, :, 2, :], flq(t_t), 4.0, flq(t_s),
                        TT.mult, TT.add)
                    # Y3 = 8u + d + U5
                    stt(t_a[:, 0:qb], t_u[:, 0:qb], 8.0, t_d[:, 0:qb],
                        TT.mult, TT.add)
                    tt(y4[:, :, 3, :], flq(t_a), flq(u5), TT.add)

                def ep_stats(co, y, ysq, cur, qb):
                    msk = mskt[cur]
                    pst = psTp.tile([64, QB], DT.float32, tag="st",
                                    name="pst")
                    psX = pst[0:32, 0:qb]
                    psXX = pst[32:64, 0:qb]
                    acg = lycp.tile([32, 512], F16, tag="acg")
                    nc.sync.dma_start(
                        acg[:], acg_d.ap()[li, :, co * 512:(co + 1) * 512])
                    gm = lycp.tile([32, 512], F16, tag="gm")
                    nc.sync.dma_start(
                        gm[:], gm_d.ap()[li, :, co * 512:(co + 1) * 512])
                    bc = lycp.tile([32, 512], F16, tag="bc")
                    nc.sync.dma_start(
                        bc[:], bc_d.ap()[li, :, co * 512:(co + 1) * 512])
                    for j in range(4):
                        nc.tensor.matmul(psX,
                                         smask[:, j * 32:(j + 1) * 32],
                                         y[:, j * qb:(j + 1) * qb],
                                         start=(j == 0), stop=(j == 3))
                    for j in range(4):
                        nc.tensor.matmul(psXX,
                                         smask[:, j * 32:(j + 1) * 32],
                                         ysq[:, j * qb:(j + 1) * qb],
                                         start=(j == 0), stop=(j == 3))

                    sxs = ttpool.tile([32, QB], DT.float32, tag="sxs")
                    nc.vector.tensor_copy(sxs[:, 0:qb], psX)
                    u2 = ttpool.tile([32, QB], DT.float32, tag="u2")
                    nc.vector.scalar_tensor_tensor(u2[:, 0:qb], sxs[:, 0:qb],
                                                   -1.0 / GSIZE, sxs[:, 0:qb],
                                                   TT.mult, TT.mult)
                    v = ttpool.tile([32, QB], DT.float32, tag="v")
                    nc.vector.tensor_tensor(v[:, 0:qb], psXX, u2[:, 0:qb],
                                            TT.add)
                    uu = ttpool.tile([32, QB], DT.float32, tag="u")
                    nc.vector.tensor_scalar(uu[:, 0:qb], v[:, 0:qb],
                                            1.0 / GSIZE, EPS,
                                            TT.mult, TT.add)
                    r = ttpool.tile([32, QB], DT.float32, tag="r")
                    nc.vector.reciprocal_approx_fast(r[:, 0:qb], uu[:, 0:qb])
                    inv = ttpool.tile([32, QB], DT.float32, tag="u2")
                    nc.scalar.activation(inv[:, 0:qb], r[:, 0:qb],
                                         mybir.ActivationFunctionType.Sqrt)
                    invm = tt2pool.tile([32, QB], F16, tag="invm")
                    nc.vector.tensor_tensor(invm[:, 0:qb], inv[:, 0:qb],
                                            msk[0:32, 0:qb], TT.mult)
                    w32 = tt2pool.tile([32, QB], F16, tag="w32")
                    nc.vector.tensor_tensor(w32[:, 0:qb], sxs[:, 0:qb],
                                            invm[:, 0:qb], TT.mult)
                    return invm, w32, msk, acg, gm, bc

                def ep_ab(co, y, invm, w32, msk, acg, gm, bc, bexpr, qb,
                          boff):
                    if final:
                        yout = yfpool.tile([128, CB], DT.float32, tag="yf")
                    else:
                        yout = y  # relu written in place after t1 reads y
                    for j in range(4):
                        cj = j * 128
                        psA = psAp2.tile([128, QB], DT.float32, tag="A",
                                         name="psA")
                        nc.tensor.matmul(psA[:, 0:qb], acg[:, cj:cj + 128],
                                         invm[:, 0:qb], start=True, stop=True)
                        psB = psBp1.tile([128, QB], DT.float32, tag="B",
                                         name="psB")
                        nc.tensor.matmul(psB[:, 0:qb], bc[:, cj:cj + 128],
                                         msk[0:32, 0:qb], start=True,
                                         stop=False)
                        nc.tensor.matmul(psB[:, 0:qb], gm[:, cj:cj + 128],
                                         w32[:, 0:qb], start=False, stop=True)
                        t1 = tt2pool.tile([128, QB], DT.float32, tag="t1")
                        nc.vector.tensor_tensor(
                            t1[:, 0:qb], psA[:, 0:qb],
                            y[:, j * qb:(j + 1) * qb], TT.mult)
                        t2 = tt2pool.tile([128, QB], DT.float32, tag="t2")
                        nc.vector.tensor_tensor(t2[:, 0:qb], psB[:, 0:qb],
                                                t1[:, 0:qb], TT.add)
                        nc.scalar.activation(
                            yout[:, j * qb:(j + 1) * qb], t2[:, 0:qb],
                            mybir.ActivationFunctionType.Relu)

                    cb = 4 * qb
                    dst = dst_aps[co][:, bass.ds(boff + (0 if final
                                                         else CPAD), cb)]
                    nc.sync.dma_start(dst, yout[:, 0:cb])

                def run_block(bexpr, pre_bexpr, cur, gq=GQ, boff=None):
                    qb = gq * SW
                    load_xw2(1 - cur, pre_bexpr, GQ)
                    v_build(cur, gq)
                    if boff is None:
                        boff = bexpr * CB
                    pstat = []
                    pab = []
                    for co in range(4):
                        y = ypool.tile([128, CB], F16, tag="y")
                        conv_transform(co, y, gq)
                        ysq = ysqpool.tile([128, CB], F16, tag="ysq")
                        nc.scalar.activation(ysq[:, 0:4 * qb],
                                             y[:, 0:4 * qb], SQ)
                        pstat.append((co, y, ysq))
                        if len(pstat) > 1:
                            c_, y_, ysq_ = pstat.pop(0)
                            st = ep_stats(c_, y_, ysq_, cur, qb)
                            pab.append((c_, y_) + st)
                        if len(pab) > 1:
                            ep_ab(*pab.pop(0), bexpr, qb, boff)
                    while pstat:
                        c_, y_, ysq_ = pstat.pop(0)
                        st = ep_stats(c_, y_, ysq_, cur, qb)
                        pab.append((c_, y_) + st)
                        while len(pab) > 1:
                            ep_ab(*pab.pop(0), bexpr, qb, boff)
                    while pab:
                        ep_ab(*pab.pop(0), bexpr, qb, boff)

                cur = 0
                load_xw2(0, 0, GQ)
                nstep = 8
                nbe = nf - (nf % nstep)
                if nbe:
                    with tc.For_i(0, nbe, nstep,
                                  hint_engines=(mybir.EngineType.PE,)) as i:
                        for u in range(nstep):
                            run_block(i + u, i + u + 1, cur)
                            cur = 1 - cur
                for t in range(nbe, nf):
                    run_block(t, t + 1 if t + 1 < nf else t, cur)
                    cur = 1 - cur
                if gt:
                    run_block(nf, nf, cur, gq=gt, boff=nf * CB)
                    cur = 1 - cur
                w_next = (load_weights(li + 1) if li + 1 < layers else None)
                tc.strict_bb_all_engine_barrier()
                return w_next

            bufs = {"x0": x0_d, "xa": xa_d, "xb": xb_d}
            seq = ["x0"] + ["xa", "xb"] * 4
            w_tiles = load_weights(0)
            for li in range(layers):
                src, dst = seq[li], seq[li + 1]
                nci = 2 if li == 0 else 4
                src_aps = [bufs[src].ap()[ci] for ci in range(nci)]
                final = li == layers - 1
                dst_aps = ([out_d.ap()[co] for co in range(4)] if final
                           else [bufs[dst].ap()[co] for co in range(4)])
                w_tiles = run_layer(li, nci, src_aps, dst_aps, final, w_tiles)

    nc.compile()
    return nc


# ------------------------------------------------------------- host packing

def _pack_host(inputs, pos, meta, layers=8):
    feats = np.ascontiguousarray(np.asarray(inputs["features"], np.float32))
    w0 = np.asarray(inputs["w0"], np.float32)
    w_rest = np.asarray(inputs["w_rest"], np.float32)
    gamma = np.asarray(inputs["gamma"], np.float32)
    beta = np.asarray(inputs["beta"], np.float32)
    n, cin = feats.shape
    stride, r8 = meta["stride"], meta["r8"]
    m_raw, m_out = meta["m_raw"], meta["m_out"]
    nf, gt = meta["n_full"], meta["g_tail"]
    rgst = meta["rg"] * stride
    win_end = CPAD + nf * CB - SW + WROWS * SW
    padw = max(CPAD + m_out, win_end) + 2 * CPAD

    x_g = np.zeros((cin, rgst), np.float16)
    x_g[:, pos] = feats.T.astype(np.float16)
    mask_g = np.zeros(rgst, np.float16)
    mask_g[pos] = 1.0

    # winograd-transformed weights: per (pos,dx) plane, cols (co, ci, ch)
    def pack_w(w, nci):
        # w: [9, Cin, 512] -> out [128, 18 * nci*4*128]
        cin_ = nci * 128
        planes = []
        for p in range(NPOS):
            for dx in range(3):
                wp = np.zeros((cin_, HID), np.float32)
                for dy in range(3):
                    wp += G4[p, dy] * w[3 * dy + dx]
                # lhsT chunks [128, 128] per (co, ci): part dim = ci part
                arr = wp.reshape(nci, 128, 4, 128).transpose(1, 2, 0, 3)
                # arr[p_part, co, ci, ch]
                planes.append(arr.reshape(128, nci * 4 * 128))
        return np.concatenate(planes, axis=1).astype(np.float16)

    w0p = pack_w(w0, 2)
    nl = max(layers - 1, 1)
    wrp = np.zeros((nl, 128, NPOS * 3 * 4 * HID), np.float16)
    for li in range(layers - 1):
        wrp[li] = pack_w(w_rest[li], 4)

    ch = np.arange(128)
    acg = np.zeros((layers, 32, 4, 4, 128), np.float32)
    gm16 = np.zeros((layers, 32, 4, 4, 128), np.float32)
    bc32 = np.zeros((layers, 32, 4, 4, 128), np.float32)
    for li in range(layers):
        for co in range(4):
            g_ = gamma[li, co * 128:(co + 1) * 128]
            b_ = beta[li, co * 128:(co + 1) * 128]
            for j in range(4):
                rows = 8 * j + ch // GSIZE
                acg[li, rows, co, j, ch] = g_
                gm16[li, rows, co, j, ch] = -g_ / GSIZE
                bc32[li, 8 * j, co, j, :] = b_
    acg = acg.reshape(layers, 32, 2048).astype(np.float16)
    gm16 = gm16.reshape(layers, 32, 2048).astype(np.float16)
    bc32 = bc32.reshape(layers, 32, 2048).astype(np.float16)

    smask = np.zeros((128, 4, 32), np.float16)
    for j in range(4):
        smask[ch, j, 8 * j + ch // GSIZE] = 1.0
    smask = smask.reshape(128, 128)

    in_maps = []
    for s in range(N_CORES):
        c0 = s * r8 * stride
        x0 = np.zeros((2, 128, padw), np.float16)
        seg = x_g[:, c0:min(c0 + m_raw, rgst)]
        x0[:, :, CPAD:CPAD + seg.shape[1]] = seg.reshape(2, 128, -1)
        nf, gt = meta["n_full"], meta["g_tail"]
        mc = np.zeros(nf * CB + 4 * gt * SW, np.float16)
        mseg = mask_g[c0:min(c0 + m_raw, rgst)]
        mc[:mseg.shape[0]] = mseg
        # full blocks: msk32[8j+g, b*QB + c] = mask[b*CB + j*QB + c]
        m4 = mc[:nf * CB].reshape(nf, 4, QB)
        msk32 = np.zeros((32, (nf + 1) * QB), np.float16)
        for j in range(4):
            for g in range(8):
                msk32[8 * j + g, :nf * QB] = m4[:, j, :].reshape(-1)
        if gt:
            qt = gt * SW
            mt = mc[nf * CB:].reshape(4, qt)
            for j in range(4):
                for g in range(8):
                    msk32[8 * j + g, nf * QB:nf * QB + qt] = mt[j]
        in_maps.append({
            "x0": x0, "w0p": w0p, "wrp": wrp, "acg": acg, "gm16": gm16,
            "bc32": bc32, "smask": smask, "msk32": msk32,
        })
    return in_maps


TRACE = False
LAST_RESULT = {}


def kernel(**inputs) -> np.ndarray:
    nbr = np.asarray(inputs["nbr_idx"])
    n = nbr.shape[0]
    pos, meta = _build_canvas_map(nbr)
    in_maps = _pack_host(inputs, pos, meta)
    nc = _build_program(meta)
    res = run_bass_kernel_spmd(nc, in_maps, list(range(N_CORES)), trace=TRACE)
    LAST_RESULT["exec_time_ns"] = res.exec_time_ns
    LAST_RESULT["profile_json"] = res.profile_json

    stride, r8 = meta["stride"], meta["r8"]
    row = pos // stride
    own = np.clip((row - HALO_ROWS) // r8, 0, N_CORES - 1)
    result = np.zeros((n, HID), np.float32)
    for s in range(N_CORES):
        sel = own == s
        local = pos[sel] - s * r8 * stride
        o = res.results[s]["out"]  # [4, 128, m_out]
        result[sel] = o[:, :, local].reshape(HID, -1).T
    return result


if __name__ == "__main__":
    import reference

    inputs = reference.setup_inputs()
    out = kernel(**{k: np.asarray(v) for k, v in inputs.items()})
    exp = np.asarray(reference.reference(**inputs))
    err = np.linalg.norm(out - exp) / np.linalg.norm(exp)
    print(f"l2 rel err: {err:.3e}")


# revision 18
# speedup vs baseline: 1.1020x; 1.0504x over previous
"""Trainium2 Bass kernel for nn_DensePoseV1ConvXGNSparseHead.

8 layers of submanifold 3x3 conv (gather-GEMM over 9 taps) + GroupNorm(32)
+ ReLU on N=131072 sparse sites, 256->512 then 512->512 channels.

Strategy
--------
Host reconstructs a planar embedding of the points from nbr_idx, packs the
connected components into a padded dense canvas (stride 66), and runs the
conv as dense channel-major fp16 matmuls.  Inactive/pad cells are kept at
exactly 0 by folding an activity mask into the GroupNorm affine, preserving
submanifold semantics.  Canvas rows are split across the 8 cores with an
8-row halo (receptive field of 8 stacked 3x3 convs) - zero inter-core
communication.

v3 (this file): the 3x3 conv runs as 1-D Winograd F(4,3) along canvas rows:
output rows in groups of 4, 6 position-planes V = B4^T x (DVE row combines),
per-position GEMMs over (3 dx taps x ci chunks) with host-transformed
weights W' = G4 w, output rows Y = A4^T U (DVE).  MACs per cell drop from
36 to 18 per 128-out-chunk, which matters because the baseline was at the
power-throttled PE streaming roofline.  GroupNorm statistics stay fp32.
"""

import numpy as np

import concourse.bass as bass
import concourse.tile as tile
from concourse import bacc, mybir
from concourse.bass_utils import run_bass_kernel_spmd

DT = mybir.dt
F16 = DT.float16

N_TAPS = 9
OFFS = [(dy, dx) for dy in (-1, 0, 1) for dx in (-1, 0, 1)]
OFFS_ARR = np.array(OFFS, np.int64)
HALO_ROWS = 8
N_CORES = 8
HID = 512
GSIZE = 16
EPS = 1e-5
CPAD = 128

SW = 66          # canvas row stride (w.max()+2, asserted host-side)
GQ = 7           # group-rows per block
QB = GQ * SW     # 462 quad-cells per block (psum width)
CB = 4 * QB      # 1848 output cells per block
WROWS = 30       # x-window rows per block (4*GQ + 2)
WPITCH = 2192    # x-window plane pitch (fits 4D rearrange span)
VW = QB + 2      # V-plane width incl 1 garbage-safe edge col each side
NPOS = 6

B4T = np.array([
    [4, 0, -5, 0, 1, 0], [0, -4, -4, 1, 1, 0], [0, 4, -4, -1, 1, 0],
    [0, -2, -1, 2, 1, 0], [0, 2, -1, -2, 1, 0], [0, 4, 0, -5, 0, 1]],
    np.float32)
G4 = np.array([
    [1 / 4, 0, 0], [-1 / 6, -1 / 6, -1 / 6], [-1 / 6, 1 / 6, -1 / 6],
    [1 / 24, 1 / 12, 1 / 6], [1 / 24, -1 / 12, 1 / 6], [0, 0, 1]],
    np.float32)
A4T = np.array([
    [1, 1, 1, 1, 1, 0], [0, 1, -1, 2, -2, 0],
    [0, 1, 1, 4, 4, 0], [0, 1, -1, 8, -8, 1]], np.float32)


# ----------------------------------------------------------------- host side

def _embed_points(nbr):
    n = nbr.shape[0]
    assert nbr.shape[1] == N_TAPS
    assert (nbr[:, 4] == np.arange(n)).all(), "tap 4 must be self"
    comp = np.arange(n, dtype=np.int64)
    py = np.zeros(n, np.int64)
    px = np.zeros(n, np.int64)
    edges = []
    for k in range(N_TAPS):
        if k == 4:
            continue
        t = nbr[:, k]
        src = np.flatnonzero(t >= 0)
        edges.append((src, t[src].astype(np.int64), int(OFFS_ARR[k, 0]),
                      int(OFFS_ARR[k, 1])))
    for _ in range(100_000):
        changed = False
        for src, dst, dy, dx in edges:
            bad = comp[src] < comp[dst]
            if bad.any():
                s, d = src[bad], dst[bad]
                order = np.argsort(comp[s], kind="stable")
                s, d = s[order], d[order]
                uniq, first = np.unique(d, return_index=True)
                s, d = s[first], uniq
                comp[d] = comp[s]
                py[d] = py[s] + dy
                px[d] = px[s] + dx
                changed = True
        if not changed:
            break
    else:
        raise RuntimeError("label propagation did not converge")
    for k in range(N_TAPS):
        t = nbr[:, k]
        src = np.flatnonzero(t >= 0)
        dst = t[src]
        ok = ((comp[src] == comp[dst])
              & (py[dst] == py[src] + OFFS_ARR[k, 0])
              & (px[dst] == px[src] + OFFS_ARR[k, 1]))
        if not ok.all():
            raise RuntimeError(f"rulebook inconsistent at tap {k}")
    return comp, py, px


def _build_canvas_map(nbr):
    n = nbr.shape[0]
    comp, py, px = _embed_points(nbr)
    uniq, inv = np.unique(comp, return_inverse=True)
    ncmp = uniq.size
    big = 1 << 60
    miny = np.full(ncmp, big); minx = np.full(ncmp, big)
    maxy = np.full(ncmp, -big); maxx = np.full(ncmp, -big)
    np.minimum.at(miny, inv, py); np.minimum.at(minx, inv, px)
    np.maximum.at(maxy, inv, py); np.maximum.at(maxx, inv, px)
    h = maxy - miny + 1
    w = maxx - minx + 1
    stride = int(w.max()) + 2
    assert stride == SW, f"stride {stride} != {SW}"
    shelf_w = stride - 2

    npts = np.bincount(inv)
    isbig = npts > 1000
    row_off = np.zeros(ncmp, np.int64)
    col_off = np.ones(ncmp, np.int64)
    acc = 0
    for c in np.flatnonzero(isbig):
        row_off[c] = acc
        acc += int(h[c]) + 1
    order = sorted(np.flatnonzero(~isbig), key=lambda c: -int(h[c]))
    shelf_row, shelf_h, xcur = acc, 0, 0
    for c in order:
        if xcur + int(w[c]) > shelf_w:
            shelf_row += shelf_h + 1
            shelf_h, xcur = 0, 0
        if shelf_h == 0:
            shelf_h = int(h[c])
        row_off[c] = shelf_row
        col_off[c] = 1 + xcur
        xcur += int(w[c]) + 1
    if xcur > 0:
        shelf_row += shelf_h + 1
    total_rows = int(shelf_row)
    # rows per core: multiple of 4 so core canvases align to quad groups
    r8 = -(-total_rows // N_CORES)
    r8 = -(-r8 // 4) * 4
    rg = N_CORES * r8 + 2 * HALO_ROWS
    grow = HALO_ROWS + row_off[inv] + (py - miny[inv])
    gcol = col_off[inv] + (px - minx[inv])
    pos = grow * stride + gcol
    occupied = np.zeros(rg * stride, bool)
    if pos.max() >= occupied.size or np.unique(pos).size != n:
        raise RuntimeError("canvas build failed")
    for k in range(N_TAPS):
        if k == 4:
            continue
        occupied[:] = False
        occupied[pos] = True
        dpos = int(OFFS_ARR[k, 0]) * stride + int(OFFS_ARR[k, 1])
        if occupied[pos[nbr[:, k] < 0] + dpos].any():
            raise RuntimeError(f"tap {k}: active cell where rulebook says -1")
    rows_core = r8 + 2 * HALO_ROWS          # 456
    assert rows_core % 4 == 0
    ng = rows_core // 4                      # quad groups per core
    nf = ng // GQ                            # full winograd blocks per core
    gt = ng - nf * GQ                        # tail block group-rows
    m_raw = rows_core * stride               # valid cells per core
    return pos, dict(stride=stride, r8=r8, rg=rg, m_raw=m_raw,
                     ng=ng, n_full=nf, g_tail=gt, m_out=m_raw)


# --------------------------------------------------------------- bass program

def _build_program(meta, layers=8):
    nf = meta["n_full"]
    gt = meta["g_tail"]
    ng = meta["ng"]
    m_raw = meta["m_raw"]
    m_out = meta["m_out"]
    # x buffers: room for the last (full-width) window read + zero slack
    win_end = CPAD + nf * CB - SW + WROWS * SW
    padw = max(CPAD + m_out, win_end) + 2 * CPAD
    z0_start = CPAD + m_raw
    nc = bacc.Bacc("TRN2", target_bir_lowering=False, debug=False)

    WTC = NPOS * 3  # 18 weight planes per layer

    x0_d = nc.dram_tensor("x0", (2, 128, padw), F16, kind="ExternalInput")
    w0_d = nc.dram_tensor("w0p", (128, WTC * 2 * HID), F16,
                          kind="ExternalInput")
    wr_d = nc.dram_tensor("wrp", (max(layers - 1, 1), 128, WTC * 4 * HID),
                          F16, kind="ExternalInput")
    acg_d = nc.dram_tensor("acg", (layers, 32, 2048), F16, kind="ExternalInput")
    gm_d = nc.dram_tensor("gm16", (layers, 32, 2048), F16, kind="ExternalInput")
    bc_d = nc.dram_tensor("bc32", (layers, 32, 2048), F16, kind="ExternalInput")
    smask_d = nc.dram_tensor("smask", (128, 128), F16, kind="ExternalInput")
    msk32_d = nc.dram_tensor("msk32", (32, (nf + 1) * QB), F16,
                             kind="ExternalInput")
    out_d = nc.dram_tensor("out", (4, 128, m_out), DT.float32,
                           kind="ExternalOutput")
    xa_d = nc.dram_tensor("xa", (4, 128, padw), F16, kind="Internal")
    xb_d = nc.dram_tensor("xb", (4, 128, padw), F16, kind="Internal")

    with tile.TileContext(nc) as tc:
        with (
            tc.tile_pool(name="consts", bufs=1) as constp,
            tc.tile_pool(name="wp", bufs=1) as wpool,
            tc.tile_pool(name="lyc", bufs=2) as lycp,
            tc.tile_pool(name="vp", bufs=1) as vpool,
            tc.tile_pool(name="vt", bufs=1) as vtp,
            tc.tile_pool(name="yb", bufs=3) as ypool,
            tc.tile_pool(name="ysq", bufs=2) as ysqpool,
            tc.tile_pool(name="yf", bufs=1) as yfpool,
            tc.tile_pool(name="ot", bufs=1) as otpool,
            tc.tile_pool(name="tt", bufs=1) as ttpool,
            tc.tile_pool(name="tt2", bufs=2) as tt2pool,
            tc.tile_pool(name="psU", bufs=1, space=bass.MemorySpace.PSUM) as psUp,
            tc.tile_pool(name="psT", bufs=2, space=bass.MemorySpace.PSUM) as psTp,
            tc.tile_pool(name="psA2", bufs=2, space=bass.MemorySpace.PSUM) as psAp2,
            tc.tile_pool(name="psB1", bufs=1, space=bass.MemorySpace.PSUM) as psBp1,
        ):
            smask = constp.tile([128, 128], F16)
            nc.sync.dma_start(smask[:], smask_d.ap())
            xw0 = constp.tile([128, 4, WPITCH], F16, tag="xw0")
            xw1 = constp.tile([128, 4, WPITCH], F16, tag="xw1")
            msk0 = constp.tile([64, QB], F16, tag="msk0")
            msk1 = constp.tile([64, QB], F16, tag="msk1")
            xwt = [xw0, xw1]
            mskt = [msk0, msk1]

            # zero the pads of the internal ping-pong buffers once
            zpad = constp.tile([128, CPAD], F16)
            nc.gpsimd.memset(zpad[:], 0.0)
            for buf in (xa_d, xb_d):
                for ci in range(4):
                    nc.sync.dma_start(buf.ap()[ci, :, 0:CPAD], zpad[:])
                    for z0 in range(z0_start, padw, CPAD):
                        zw = min(CPAD, padw - z0)
                        nc.sync.dma_start(buf.ap()[ci, :, z0:z0 + zw],
                                          zpad[:, 0:zw])

            # V plane tiles: fixed pos tags holding all 4 ci planes; edge
            # cols written once and never again (only reach masked outputs)
            vtiles = {}
            for p in range(NPOS):
                vtiles[p] = vpool.tile([128, 4, VW], F16, tag=f"v{p}",
                                       name=f"v{p}")
            for v in vtiles.values():
                for ci in range(4):
                    nc.gpsimd.memset(v[:, ci, 0:1], 0.0)
                    nc.gpsimd.memset(v[:, ci, VW - 1:VW], 0.0)

            def load_weights(li):
                # 18 (pos,dx) planes; tag-level deps let the next layer's
                # DMAs start as this layer's last reader of a plane retires
                nci = 2 if li == 0 else 4
                wq = nci * 4 * 128
                tiles = []
                for pd in range(WTC):
                    wsb = wpool.tile([128, 4 * 4 * 128], F16, tag=f"w{pd}",
                                     name=f"w{pd}")
                    src = (w0_d.ap() if li == 0 else wr_d.ap()[li - 1])
                    nc.sync.dma_start(wsb[:, 0:wq],
                                      src[:, pd * wq:(pd + 1) * wq])
                    tiles.append(wsb)
                return tiles

            def load_xw(pp, src_aps, nci, bexpr):
                for ci in range(nci):
                    nc.sync.dma_start(
                        xwt[pp][:, ci, 0:WROWS * SW],
                        src_aps[ci][:, bass.ds(bexpr * CB + (CPAD - SW),
                                               WROWS * SW)])
                nc.sync.dma_start(
                    mskt[pp][0:32, :],
                    msk32_d.ap()[:, bass.ds(bexpr * QB, QB)])
                nc.sync.dma_start(
                    mskt[pp][32:64, :],
                    msk32_d.ap()[:, bass.ds(bexpr * QB, QB)])

            TT = mybir.AluOpType
            SQ = mybir.ActivationFunctionType.Square
            CPF = mybir.ActivationFunctionType.Copy

            def run_layer(li, nci, src_aps, dst_aps, final, w_tiles):

                def load_xw2(pp, bexpr, gq):
                    wr = 4 * gq + 2
                    for ci in range(nci):
                        nc.sync.dma_start(
                            xwt[pp][:, ci, 0:wr * SW],
                            src_aps[ci][:, bass.ds(bexpr * CB + (CPAD - SW),
                                                   wr * SW)])
                    qb = gq * SW
                    nc.sync.dma_start(
                        mskt[pp][0:32, 0:qb],
                        msk32_d.ap()[:, bass.ds(bexpr * QB, qb)])
                    nc.sync.dma_start(
                        mskt[pp][32:64, 0:qb],
                        msk32_d.ap()[:, bass.ds(bexpr * QB, qb)])

                def v_build(cur, gq):
                    xw = xwt[cur]
                    qb = gq * SW

                    def d(i):
                        seg = xw[:, 0:nci, i * SW:i * SW + gq * 4 * SW]
                        return seg.rearrange("p ci (g q c) -> p ci g q c",
                                             g=gq, q=4)[:, :, :, 0, :]

                    tt = nc.vector.tensor_tensor
                    stt = nc.vector.scalar_tensor_tensor

                    def sc(out, in_, k):
                        nc.scalar.activation(out, in_, CPF, scale=float(k))

                    def fl(t):
                        return t[:, 0:4 * qb].rearrange(
                            "p (ci g c) -> p ci g c",
                            ci=4, g=gq)[:, 0:nci]

                    def f3(t):
                        return t[:, 0:4 * qb].rearrange(
                            "p (ci q) -> p ci q", ci=4)[:, 0:nci]

                    def vout(p):
                        return vtiles[p][:, 0:nci, 1:1 + qb].rearrange(
                            "p ci (g c) -> p ci g c", g=gq)

                    def vo3(p):
                        return vtiles[p][:, 0:nci, 1:1 + qb]

                    d0, d1, d2, d3, d4, d5 = (d(i) for i in range(6))
                    s1 = vtp.tile([128, 4 * QB], F16, tag="s1", name="vt_s1")
                    s2 = vtp.tile([128, 4 * QB], F16, tag="s2", name="vt_s2")
                    # V1 = (d1+d2)*(-4) + (d3+d4); V2 = (d1-d2)*4 + (d4-d3)
                    tt(fl(s1), d1, d2, TT.add)
                    tt(fl(s2), d3, d4, TT.add)
                    stt(vo3(1), f3(s1), -4.0, f3(s2), TT.mult, TT.add)
                    tt(fl(s1), d1, d2, TT.subtract)
                    tt(fl(s2), d4, d3, TT.subtract)
                    stt(vo3(2), f3(s1), 4.0, f3(s2), TT.mult, TT.add)
                    # V3 = 2(d3-d1) + (d4-d2); V4 = -2(d3-d1) + (d4-d2)
                    tt(fl(s1), d3, d1, TT.subtract)
                    tt(fl(s2), d4, d2, TT.subtract)
                    stt(vo3(3), f3(s1), 2.0, f3(s2), TT.mult, TT.add)
                    stt(vo3(4), f3(s1), -2.0, f3(s2), TT.mult, TT.add)
                    # V0 = 4 d0 + (-5 d2 + d4);  V5 = 4 d1 + (-5 d3 + d5)
                    sc(fl(s1), d2, -5.0)
                    tt(fl(s2), fl(s1), d4, TT.add)
                    sc(fl(s1), d0, 4.0)
                    tt(vout(0), fl(s1), fl(s2), TT.add)
                    sc(fl(s1), d3, -5.0)
                    tt(fl(s2), fl(s1), d5, TT.add)
                    sc(fl(s1), d1, 4.0)
                    tt(vout(5), fl(s1), fl(s2), TT.add)

                def chain(pos, co, psu, qb):
                    mi = 0
                    nmm = 3 * nci
                    for dx in range(3):
                        wt = w_tiles[pos * 3 + dx]
                        for ci in range(nci):
                            lhsT = wt[:, (co * nci + ci) * 128:
                                      (co * nci + ci) * 128 + 128]
                            rhs = vtiles[pos][:, ci, dx:dx + qb]
                            nc.tensor.matmul(psu[:, 0:qb], lhsT, rhs,
                                             start=(mi == 0),
                                             stop=(mi == nmm - 1))
                            mi += 1

                def conv_transform(co, y, gq):
                    # 6 position chains through 3 rotating psum banks,
                    # interleaved with the A4T output transform.  Tag roles
                    # rotate per co so the next co's first chain reuses the
                    # bank that was freed earliest.
                    qb = gq * SW
                    tt = nc.vector.tensor_tensor
                    stt = nc.vector.scalar_tensor_tensor
                    r1, r2, r3 = (f"U{(co + k) % 3}" for k in range(3))

                    def utile(tag, nm):
                        return psUp.tile([128, QB], DT.float32, tag=tag,
                                         name=nm)

                    ua = utile(r1, "psUa")
                    chain(1, co, ua, qb)
                    ub = utile(r2, "psUb")
                    chain(2, co, ub, qb)
                    c1 = otpool.tile([128, QB], DT.float32, tag="oc",
                                     name="ot_c")
                    nc.vector.tensor_copy(c1[:, 0:qb], ua[:, 0:qb])
                    t_s = otpool.tile([128, QB], DT.float32, tag="os",
                                      name="ot_s")
                    tt(t_s[:, 0:qb], c1[:, 0:qb], ub[:, 0:qb], TT.add)
                    t_d = otpool.tile([128, QB], DT.float32, tag="od",
                                      name="ot_d")
                    tt(t_d[:, 0:qb], c1[:, 0:qb], ub[:, 0:qb], TT.subtract)
                    uc = utile(r1, "psUc")
                    chain(3, co, uc, qb)
                    ud = utile(r3, "psUd")
                    chain(4, co, ud, qb)
                    c2 = otpool.tile([128, QB], DT.float32, tag="oc",
                                     name="ot_c2")
                    nc.vector.tensor_copy(c2[:, 0:qb], uc[:, 0:qb])
                    t_t = otpool.tile([128, QB], DT.float32, tag="ost",
                                      name="ot_t")
                    tt(t_t[:, 0:qb], c2[:, 0:qb], ud[:, 0:qb], TT.add)
                    t_u = otpool.tile([128, QB], DT.float32, tag="ou",
                                      name="ot_u")
                    tt(t_u[:, 0:qb], c2[:, 0:qb], ud[:, 0:qb], TT.subtract)
                    u5 = utile(r1, "psU5")
                    chain(5, co, u5, qb)
                    u0 = utile(r2, "psU0")
                    chain(0, co, u0, qb)

                    y4 = y[:, 0:4 * qb].rearrange("p (g r c) -> p g r c",
                                                  g=gq, r=4)

                    def flq(ap):
                        return ap[:, 0:qb].rearrange("p (g c) -> p g c", g=gq)

                    t_a = otpool.tile([128, QB], DT.float32, tag="oa",
                                      name="ot_a")
                    # Y3 = 8u + d + U5   (emitted first: frees U5's bank)
                    stt(t_a[:, 0:qb], t_u[:, 0:qb], 8.0, t_d[:, 0:qb],
                        TT.mult, TT.add)
                    tt(y4[:, :, 3, :], flq(t_a), flq(u5), TT.add)
                    # Y1 = 2u + d ; Y2 = 4t + s  (no U deps)
                    stt(y4[:, :, 1, :], flq(t_u), 2.0, flq(t_d),
                        TT.mult, TT.add)
                    stt(y4[:, :, 2, :], flq(t_t), 4.0, flq(t_s),
                        TT.mult, TT.add)
                    # Y0 = U0 + s + t   (last)
                    tt(t_a[:, 0:qb], u0[:, 0:qb], t_s[:, 0:qb], TT.add)
                    tt(y4[:, :, 0, :], flq(t_a), flq(t_t), TT.add)

                def ep_stats(co, y, ysq, cur, qb):
                    msk = mskt[cur]
                    pst = psTp.tile([64, QB], DT.float32, tag="st",
                                    name="pst")
                    psX = pst[0:32, 0:qb]
                    psXX = pst[32:64, 0:qb]
                    acg = lycp.tile([32, 512], F16, tag="acg")
                    nc.sync.dma_start(
                        acg[:], acg_d.ap()[li, :, co * 512:(co + 1) * 512])
                    gm = lycp.tile([32, 512], F16, tag="gm")
                    nc.sync.dma_start(
                        gm[:], gm_d.ap()[li, :, co * 512:(co + 1) * 512])
                    bc = lycp.tile([32, 512], F16, tag="bc")
                    nc.sync.dma_start(
                        bc[:], bc_d.ap()[li, :, co * 512:(co + 1) * 512])
                    for j in range(4):
                        nc.tensor.matmul(psX,
                                         smask[:, j * 32:(j + 1) * 32],
                                         y[:, j * qb:(j + 1) * qb],
                                         start=(j == 0), stop=(j == 3))
                    for j in range(4):
                        nc.tensor.matmul(psXX,
                                         smask[:, j * 32:(j + 1) * 32],
                                         ysq[:, j * qb:(j + 1) * qb],
                                         start=(j == 0), stop=(j == 3))

                    sxs = ttpool.tile([32, QB], DT.float32, tag="sxs")
                    nc.vector.tensor_copy(sxs[:, 0:qb], psX)
                    u2 = ttpool.tile([32, QB], DT.float32, tag="u2")
                    nc.vector.scalar_tensor_tensor(u2[:, 0:qb], sxs[:, 0:qb],
                                                   -1.0 / GSIZE, sxs[:, 0:qb],
                                                   TT.mult, TT.mult)
                    v = ttpool.tile([32, QB], DT.float32, tag="v")
                    nc.vector.tensor_tensor(v[:, 0:qb], psXX, u2[:, 0:qb],
                                            TT.add)
                    uu = ttpool.tile([32, QB], DT.float32, tag="u")
                    nc.vector.tensor_scalar(uu[:, 0:qb], v[:, 0:qb],
                                            1.0 / GSIZE, EPS,
                                            TT.mult, TT.add)
                    r = ttpool.tile([32, QB], DT.float32, tag="r")
                    nc.vector.reciprocal_approx_fast(r[:, 0:qb], uu[:, 0:qb])
                    inv = ttpool.tile([32, QB], DT.float32, tag="u2")
                    nc.scalar.activation(inv[:, 0:qb], r[:, 0:qb],
                                         mybir.ActivationFunctionType.Sqrt)
                    invm = tt2pool.tile([32, QB], F16, tag="invm")
                    nc.vector.tensor_tensor(invm[:, 0:qb], inv[:, 0:qb],
                                            msk[0:32, 0:qb], TT.mult)
                    w32 = tt2pool.tile([32, QB], F16, tag="w32")
                    nc.vector.tensor_tensor(w32[:, 0:qb], sxs[:, 0:qb],
                                            invm[:, 0:qb], TT.mult)
                    return invm, w32, msk, acg, gm, bc

                def ep_ab(co, y, invm, w32, msk, acg, gm, bc, bexpr, qb,
                          boff):
                    if final:
                        yout = yfpool.tile([128, CB], DT.float32, tag="yf")
                    else:
                        yout = y  # relu written in place after t1 reads y
                    for j in range(4):
                        cj = j * 128
                        psA = psAp2.tile([128, QB], DT.float32, tag="A",
                                         name="psA")
                        nc.tensor.matmul(psA[:, 0:qb], acg[:, cj:cj + 128],
                                         invm[:, 0:qb], start=True, stop=True)
                        psB = psBp1.tile([128, QB], DT.float32, tag="B",
                                         name="psB")
                        nc.tensor.matmul(psB[:, 0:qb], bc[:, cj:cj + 128],
                                         msk[0:32, 0:qb], start=True,
                                         stop=False)
                        nc.tensor.matmul(psB[:, 0:qb], gm[:, cj:cj + 128],
                                         w32[:, 0:qb], start=False, stop=True)
                        t1 = tt2pool.tile([128, QB], DT.float32, tag="t1")
                        nc.vector.tensor_tensor(
                            t1[:, 0:qb], psA[:, 0:qb],
                            y[:, j * qb:(j + 1) * qb], TT.mult)
                        t2 = tt2pool.tile([128, QB], DT.float32, tag="t2")
                        nc.vector.tensor_tensor(t2[:, 0:qb], psB[:, 0:qb],
                                                t1[:, 0:qb], TT.add)
                        nc.scalar.activation(
                            yout[:, j * qb:(j + 1) * qb], t2[:, 0:qb],
                            mybir.ActivationFunctionType.Relu)

                    cb = 4 * qb
                    dst = dst_aps[co][:, bass.ds(boff + (0 if final
                                                         else CPAD), cb)]
                    nc.sync.dma_start(dst, yout[:, 0:cb])

                def run_block(bexpr, pre_bexpr, cur, gq=GQ, boff=None):
                    qb = gq * SW
                    load_xw2(1 - cur, pre_bexpr, GQ)
                    v_build(cur, gq)
                    if boff is None:
                        boff = bexpr * CB
                    pstat = []
                    pab = []
                    for co in range(4):
                        y = ypool.tile([128, CB], F16, tag="y")
                        conv_transform(co, y, gq)
                        ysq = ysqpool.tile([128, CB], F16, tag="ysq")
                        nc.scalar.activation(ysq[:, 0:4 * qb],
                                             y[:, 0:4 * qb], SQ)
                        pstat.append((co, y, ysq))
                        if len(pstat) > 1:
                            c_, y_, ysq_ = pstat.pop(0)
                            st = ep_stats(c_, y_, ysq_, cur, qb)
                            pab.append((c_, y_) + st)
                        if len(pab) > 1:
                            ep_ab(*pab.pop(0), bexpr, qb, boff)
                    while pstat:
                        c_, y_, ysq_ = pstat.pop(0)
                        st = ep_stats(c_, y_, ysq_, cur, qb)
                        pab.append((c_, y_) + st)
                        while len(pab) > 1:
                            ep_ab(*pab.pop(0), bexpr, qb, boff)
                    while pab:
                        ep_ab(*pab.pop(0), bexpr, qb, boff)

                cur = 0
                load_xw2(0, 0, GQ)
                nstep = 8
                nbe = nf - (nf % nstep)
                if nbe:
                    with tc.For_i(0, nbe, nstep,
                                  hint_engines=(mybir.EngineType.PE,)) as i:
                        for u in range(nstep):
                            run_block(i + u, i + u + 1, cur)
                            cur = 1 - cur
                for t in range(nbe, nf):
                    run_block(t, t + 1 if t + 1 < nf else t, cur)
                    cur = 1 - cur
                if gt:
                    run_block(nf, nf, cur, gq=gt, boff=nf * CB)
                    cur = 1 - cur
                w_next = (load_weights(li + 1) if li + 1 < layers else None)
                tc.strict_bb_all_engine_barrier()
                return w_next

            bufs = {"x0": x0_d, "xa": xa_d, "xb": xb_d}
            seq = ["x0"] + ["xa", "xb"] * 4
            w_tiles = load_weights(0)
            for li in range(layers):
                src, dst = seq[li], seq[li + 1]
                nci = 2 if li == 0 else 4
                src_aps = [bufs[src].ap()[ci] for ci in range(nci)]
                final = li == layers - 1
                dst_aps = ([out_d.ap()[co] for co in range(4)] if final
                           else [bufs[dst].ap()[co] for co in range(4)])
                w_tiles = run_layer(li, nci, src_aps, dst_aps, final, w_tiles)

    nc.compile()
    return nc


# ------------------------------------------------------------- host packing

def _pack_host(inputs, pos, meta, layers=8):
    feats = np.ascontiguousarray(np.asarray(inputs["features"], np.float32))
    w0 = np.asarray(inputs["w0"], np.float32)
    w_rest = np.asarray(inputs["w_rest"], np.float32)
    gamma = np.asarray(inputs["gamma"], np.float32)
    beta = np.asarray(inputs["beta"], np.float32)
    n, cin = feats.shape
    stride, r8 = meta["stride"], meta["r8"]
    m_raw, m_out = meta["m_raw"], meta["m_out"]
    nf, gt = meta["n_full"], meta["g_tail"]
    rgst = meta["rg"] * stride
    win_end = CPAD + nf * CB - SW + WROWS * SW
    padw = max(CPAD + m_out, win_end) + 2 * CPAD

    x_g = np.zeros((cin, rgst), np.float16)
    x_g[:, pos] = feats.T.astype(np.float16)
    mask_g = np.zeros(rgst, np.float16)
    mask_g[pos] = 1.0

    # winograd-transformed weights: per (pos,dx) plane, cols (co, ci, ch)
    def pack_w(w, nci):
        # w: [9, Cin, 512] -> out [128, 18 * nci*4*128]
        cin_ = nci * 128
        planes = []
        for p in range(NPOS):
            for dx in range(3):
                wp = np.zeros((cin_, HID), np.float32)
                for dy in range(3):
                    wp += G4[p, dy] * w[3 * dy + dx]
                # lhsT chunks [128, 128] per (co, ci): part dim = ci part
                arr = wp.reshape(nci, 128, 4, 128).transpose(1, 2, 0, 3)
                # arr[p_part, co, ci, ch]
                planes.append(arr.reshape(128, nci * 4 * 128))
        return np.concatenate(planes, axis=1).astype(np.float16)

    w0p = pack_w(w0, 2)
    nl = max(layers - 1, 1)
    wrp = np.zeros((nl, 128, NPOS * 3 * 4 * HID), np.float16)
    for li in range(layers - 1):
        wrp[li] = pack_w(w_rest[li], 4)

    ch = np.arange(128)
    acg = np.zeros((layers, 32, 4, 4, 128), np.float32)
    gm16 = np.zeros((layers, 32, 4, 4, 128), np.float32)
    bc32 = np.zeros((layers, 32, 4, 4, 128), np.float32)
    for li in range(layers):
        for co in range(4):
            g_ = gamma[li, co * 128:(co + 1) * 128]
            b_ = beta[li, co * 128:(co + 1) * 128]
            for j in range(4):
                rows = 8 * j + ch // GSIZE
                acg[li, rows, co, j, ch] = g_
                gm16[li, rows, co, j, ch] = -g_ / GSIZE
                bc32[li, 8 * j, co, j, :] = b_
    acg = acg.reshape(layers, 32, 2048).astype(np.float16)
    gm16 = gm16.reshape(layers, 32, 2048).astype(np.float16)
    bc32 = bc32.reshape(layers, 32, 2048).astype(np.float16)

    smask = np.zeros((128, 4, 32), np.float16)
    for j in range(4):
        smask[ch, j, 8 * j + ch // GSIZE] = 1.0
    smask = smask.reshape(128, 128)

    in_maps = []
    for s in range(N_CORES):
        c0 = s * r8 * stride
        x0 = np.zeros((2, 128, padw), np.float16)
        seg = x_g[:, c0:min(c0 + m_raw, rgst)]
        x0[:, :, CPAD:CPAD + seg.shape[1]] = seg.reshape(2, 128, -1)
        nf, gt = meta["n_full"], meta["g_tail"]
        mc = np.zeros(nf * CB + 4 * gt * SW, np.float16)
        mseg = mask_g[c0:min(c0 + m_raw, rgst)]
        mc[:mseg.shape[0]] = mseg
        # full blocks: msk32[8j+g, b*QB + c] = mask[b*CB + j*QB + c]
        m4 = mc[:nf * CB].reshape(nf, 4, QB)
        msk32 = np.zeros((32, (nf + 1) * QB), np.float16)
        for j in range(4):
            for g in range(8):
                msk32[8 * j + g, :nf * QB] = m4[:, j, :].reshape(-1)
        if gt:
            qt = gt * SW
            mt = mc[nf * CB:].reshape(4, qt)
            for j in range(4):
                for g in range(8):
                    msk32[8 * j + g, nf * QB:nf * QB + qt] = mt[j]
        in_maps.append({
            "x0": x0, "w0p": w0p, "wrp": wrp, "acg": acg, "gm16": gm16,
            "bc32": bc32, "smask": smask, "msk32": msk32,
        })
    return in_maps


TRACE = False
LAST_RESULT = {}


def kernel(**inputs) -> np.ndarray:
    nbr = np.asarray(inputs["nbr_idx"])
    n = nbr.shape[0]
    pos, meta = _build_canvas_map(nbr)
    in_maps = _pack_host(inputs, pos, meta)
    nc = _build_program(meta)
    res = run_bass_kernel_spmd(nc, in_maps, list(range(N_CORES)), trace=TRACE)
    LAST_RESULT["exec_time_ns"] = res.exec_time_ns
    LAST_RESULT["profile_json"] = res.profile_json

    stride, r8 = meta["stride"], meta["r8"]
    row = pos // stride
    own = np.clip((row - HALO_ROWS) // r8, 0, N_CORES - 1)
    result = np.zeros((n, HID), np.float32)
    for s in range(N_CORES):
        sel = own == s
        local = pos[sel] - s * r8 * stride
        o = res.results[s]["out"]  # [4, 128, m_out]
        result[sel] = o[:, :, local].reshape(HID, -1).T
    return result


if __name__ == "__main__":
    import reference

    inputs = reference.setup_inputs()
    out = kernel(**{k: np.asarray(v) for k, v in inputs.items()})
    exp = np.asarray(reference.reference(**inputs))
    err = np.linalg.norm(out - exp) / np.linalg.norm(exp)
    print(f"l2 rel err: {err:.3e}")
